# revision 1
# baseline (speedup 1.0000x reference)
# Trainium2 Bass kernel for nn_DeformSpaceAttentionv2 (deformable 3x3 max-
# sampling attention). Self-contained: hardcodes all shapes/sharding.
#
# Math: the whole channel pipeline after the deformable-unfold-max collapses
# to logits = A @ feat + c0 with A = W1*diag(gamma/sqrt(var+eps))*W0 (4x256),
# so per pixel we need feat[c] = max_k bilinear_k(x)[c], then a 4-way
# contraction, sigmoid, and channel-tiling.
#
# Sharding: 8 cores = batch (2) x 32-row bands (4). Each core:
#  - computes bilinear corner weights, validity masks and gather indices on
#    the Vector engine (floor via verified round-to-nearest cast tricks),
#  - gathers all FOUR bilinear corners in one canonical one-index-per-
#    partition indirect_dma_start from a precomputed 4-corner neighborhood
#    table in HBM (T[s] = x-channels at positions s, s+1, s+128, s+129 of
#    the zero-padded image; 1024 bf16 elems/row, 34MB/batch), so only
#    288 gathers/core are needed (9 kernel points x 32 rows),
#  - fused multiply-accumulate (scalar_tensor_tensor) with per-partition
#    (pixel) weights in bf16, running max over the 9 samples,
#  - A-contraction via tensor-multiply + tensor_reduce, 32x32 DVE
#    transposes, ACT sigmoid, 64x channel-replicated store to NCHW.
#
# NOTE: dma_gather / multi-index indirect DMA compile but execute
# incorrectly through this PJRT path (probed); only the [128,1]-index
# indirect form is used.
import numpy as np
import ml_dtypes

import concourse.bass as bass
import concourse.tile as tile
from concourse import mybir
from concourse.bass_utils import run_bass_kernel_spmd
from concourse.masks import make_identity

BN_EPS = 1e-5
B, C, H, W = 2, 256, 128, 128
G4 = 4
ROWS = 32            # output rows per core
NCORES = 8
NPOS = H * W         # 16384
NK = 9
BLKS = 2             # 16-row gather blocks per core
BLKROWS = 16

f32 = mybir.dt.float32
bf16 = mybir.dt.bfloat16
i16 = mybir.dt.int16
i32 = mybir.dt.int32

_prog_cache = {}


def _split_waits(nc, max_waits=1):
    """walrus codegen supports only 1 sem-wait per instruction; split extras
    onto preceding NoOps."""
    for bb in nc.m.functions[0].blocks:
        new_insts = []
        for ins in bb.instructions:
            si = ins.sync_info
            if si is not None and si.on_wait and len(si.on_wait) > max_waits:
                waits = list(si.on_wait)
                extra, keep = waits[:-max_waits], waits[-max_waits:]
                for i in range(0, len(extra), max_waits):
                    chunk = extra[i:i + max_waits]
                    nop = mybir.InstNoOp(name=f"{ins.name}-wsplit-{i}", ins=[], outs=[])
                    nop.engine = ins.engine
                    nop.sync_info = mybir.SyncInfo(on_wait=chunk, on_update=[])
                    new_insts.append(nop)
                si.on_wait = keep
            new_insts.append(ins)
        bb.instructions[:] = new_insts


def _build_program(no_gather=False, no_macs=False, no_tail=False, GBUFS=2, DVE_Y=16):
    nc = bass.Bass("TRN2", target_bir_lowering=False)

    xf = nc.declare_dram_parameter("xf", [16788, 4 * C], bf16, isOutput=False)
    offp = nc.declare_dram_parameter("offp", [128, NK * ROWS * 2], f32, isOutput=False)
    yk = nc.declare_dram_parameter("yk", [128, NK * ROWS], f32, isOutput=False)
    xk = nc.declare_dram_parameter("xk", [128, NK * ROWS], f32, isOutput=False)
    xg = nc.declare_dram_parameter("xg", [128, 1], f32, isOutput=False)
    arep = nc.declare_dram_parameter("arep", [128, G4 * C], bf16, isOutput=False)
    c0p = nc.declare_dram_parameter("c0p", [128, G4], f32, isOutput=False)
    c0xp = nc.declare_dram_parameter("c0xp", [128, G4 * BLKROWS], f32, isOutput=False)
    out = nc.declare_dram_parameter("out", [C, ROWS, W], f32, isOutput=True)


    NC_ = NK * ROWS          # 288 weight columns
    with tile.TileContext(nc) as tc:
        with (
            tc.tile_pool(name="consts", bufs=1) as consts,
            tc.tile_pool(name="wchain", bufs=1) as wchain,
            tc.tile_pool(name="gpool", bufs=GBUFS) as gpool,
            tc.tile_pool(name="mpool", bufs=2) as mpool,
            tc.tile_pool(name="spool", bufs=3) as spool,
            tc.tile_pool(name="opool", bufs=2) as opool,
            tc.tile_pool(name="psum", bufs=2, space="PSUM") as psum_pool,
        ):
            # ---- const loads ----
            offp_sb = consts.tile([128, NC_ * 2], f32)
            nc.sync.dma_start(out=offp_sb, in_=offp[:, :])
            yk_sb = consts.tile([128, NC_], f32)
            nc.sync.dma_start(out=yk_sb, in_=yk[:, :])
            xk_sb = consts.tile([128, NC_], f32)
            nc.sync.dma_start(out=xk_sb, in_=xk[:, :])
            xg_sb = consts.tile([128, 1], f32)
            nc.sync.dma_start(out=xg_sb, in_=xg[:, :])
            arep_sb = consts.tile([128, G4 * C], bf16)
            nc.sync.dma_start(out=arep_sb, in_=arep[:, :])
            c0_sb = consts.tile([128, G4], f32)
            nc.sync.dma_start(out=c0_sb, in_=c0p[:, :])
            c0x_sb = consts.tile([128, G4 * BLKROWS], f32)
            nc.sync.dma_start(out=c0x_sb, in_=c0xp[:, :])
            id_sb = consts.tile([128, 128], bf16)
            make_identity(nc, id_sb[:])

            Alu = mybir.AluOpType
            TT = nc.vector.tensor_tensor
            TS = nc.vector.tensor_scalar
            STT = nc.vector.scalar_tensor_tensor

            def wt(name):
                return wchain.tile([128, NC_], f32, tag=name, name=name)

            # ---- weight / index chain (all [128, 288] f32) ----
            off_y = offp_sb[:].rearrange("p (m c) -> p m c", c=2)[:, :, 0]
            off_x = offp_sb[:].rearrange("p (m c) -> p m c", c=2)[:, :, 1]

            ty = wt("ty"); TT(out=ty, in0=off_y, in1=yk_sb[:], op=Alu.add)      # py-0.5
            tx0 = wt("tx0"); TT(out=tx0, in0=off_x, in1=xk_sb[:], op=Alu.add)
            tx = wt("tx"); TS(out=tx, in0=tx0, scalar1=xg_sb[:, 0:1], scalar2=None, op0=Alu.add)  # px-0.5

            y0i = wchain.tile([128, NC_], i32, tag="y0i", name="y0i")
            nc.vector.tensor_copy(out=y0i, in_=ty)                             # rne(py-0.5)=floor(py)
            y0f = wt("y0f"); nc.vector.tensor_copy(out=y0f, in_=y0i)
            fy = wt("fy"); STT(out=fy, in0=ty, scalar=0.5, in1=y0f, op0=Alu.add, op1=Alu.subtract)

            x0i = wchain.tile([128, NC_], i32, tag="x0i", name="x0i")
            nc.vector.tensor_copy(out=x0i, in_=tx)
            x0f = wt("x0f"); nc.vector.tensor_copy(out=x0f, in_=x0i)
            fx = wt("fx"); STT(out=fx, in0=tx, scalar=0.5, in1=x0f, op0=Alu.add, op1=Alu.subtract)

            y0c = wt("y0c"); TS(out=y0c, in0=y0f, scalar1=0.0, scalar2=127.0, op0=Alu.max, op1=Alu.min)
            v0 = wt("v0"); TT(out=v0, in0=y0f, in1=y0c, op=Alu.is_equal)
            y1f = wt("y1f"); TS(out=y1f, in0=y0f, scalar1=1.0, scalar2=None, op0=Alu.add)
            y1c = wt("y1c"); TS(out=y1c, in0=y1f, scalar1=0.0, scalar2=127.0, op0=Alu.max, op1=Alu.min)
            v1 = wt("v1"); TT(out=v1, in0=y1f, in1=y1c, op=Alu.is_equal)

            xc0 = wt("xc0"); TS(out=xc0, in0=x0f, scalar1=0.0, scalar2=127.0, op0=Alu.max, op1=Alu.min)
            vx0 = wt("vx0"); TT(out=vx0, in0=x0f, in1=xc0, op=Alu.is_equal)
            x1f = wt("x1f"); TS(out=x1f, in0=x0f, scalar1=1.0, scalar2=None, op0=Alu.add)
            xc1 = wt("xc1"); TS(out=xc1, in0=x1f, scalar1=0.0, scalar2=127.0, op0=Alu.max, op1=Alu.min)
            vx1 = wt("vx1"); TT(out=vx1, in0=x1f, in1=xc1, op=Alu.is_equal)


            wy0 = wt("wy0"); TS(out=wy0, in0=fy, scalar1=-1.0, scalar2=1.0, op0=Alu.mult, op1=Alu.add)
            wy0v = wt("wy0v"); TT(out=wy0v, in0=wy0, in1=v0, op=Alu.mult)
            wy1v = wt("wy1v"); TT(out=wy1v, in0=fy, in1=v1, op=Alu.mult)
            wx0 = wt("wx0"); TS(out=wx0, in0=fx, scalar1=-1.0, scalar2=1.0, op0=Alu.mult, op1=Alu.add)
            wx0v = wt("wx0v"); TT(out=wx0v, in0=wx0, in1=vx0, op=Alu.mult)
            wx1v = wt("wx1v"); TT(out=wx1v, in0=fx, in1=vx1, op=Alu.mult)

            w00 = wt("w00"); TT(out=w00, in0=wy0v, in1=wx0v, op=Alu.mult)
            w01 = wt("w01"); TT(out=w01, in0=wy0v, in1=wx1v, op=Alu.mult)
            w10 = wt("w10"); TT(out=w10, in0=wy1v, in1=wx0v, op=Alu.mult)
            w11 = wt("w11"); TT(out=w11, in0=wy1v, in1=wx1v, op=Alu.mult)

            xb = wt("xb"); TS(out=xb, in0=x0f, scalar1=-1.0, scalar2=128.0, op0=Alu.max, op1=Alu.min)
            xb1 = wt("xb1"); TS(out=xb1, in0=xb, scalar1=1.0, scalar2=None, op0=Alu.add)
            y0s = wt("y0s"); TS(out=y0s, in0=y0f, scalar1=-2.0, scalar2=128.0, op0=Alu.max, op1=Alu.min)
            xb2 = wt("xb2"); TS(out=xb2, in0=xb1, scalar1=256.0, scalar2=None, op0=Alu.add)
            sf = wt("sf"); STT(out=sf, in0=y0s, scalar=128.0, in1=xb2, op0=Alu.mult, op1=Alu.add)
            sidx = wchain.tile([128, NC_], i32, tag="sidx", name="sidx")
            nc.vector.tensor_copy(out=sidx, in_=sf)

            # ---- main loop ----
            for blk in range(BLKS):
                m = (mpool.tile([128, BLKROWS, C], bf16, tag="m", name="m")
                     if not no_macs else None)
                for k in range(NK):
                    csl = slice(k * ROWS + blk * BLKROWS, k * ROWS + blk * BLKROWS + BLKROWS)
                    Gq = gpool.tile([128, BLKROWS, 4 * C], bf16, tag="Gq", name="Gq")
                    for y in range(BLKROWS):
                        col = k * ROWS + blk * BLKROWS + y
                        nc.gpsimd.indirect_dma_start(
                            out=Gq[:, y, :], out_offset=None, in_=xf[:, :],
                            in_offset=bass.IndirectOffsetOnAxis(
                                ap=sidx[:, col:col + 1], axis=0))
                    if not no_macs:
                        # corner-major emission: 16 independent chains (one
                        # per y). Rows >= DVE_Y go to GPSIMD (otherwise idle)
                        # to balance engine load.
                        def eng(y):
                            return nc.vector
                        tgts = []
                        for y in range(BLKROWS):
                            if k == 0:
                                tgts.append(m[:, y, :])
                            else:
                                tgts.append(spool.tile([128, C], bf16, tag=f"s{y % 6}",
                                                       name=f"s{y % 6}"))
                        for y in range(BLKROWS):
                            col = k * ROWS + blk * BLKROWS + y
                            eng(y).tensor_scalar(out=tgts[y], in0=Gq[:, y, 0:C],
                               scalar1=w00[:, col:col + 1], scalar2=None, op0=Alu.mult)
                        for (rr, cc, wti) in ((0, 1, w01), (1, 0, w10), (1, 1, w11)):
                            for y in range(BLKROWS):
                                col = k * ROWS + blk * BLKROWS + y
                                eng(y).scalar_tensor_tensor(
                                    out=tgts[y], in0=Gq[:, y, (rr * 2 + cc) * C:(rr * 2 + cc + 1) * C],
                                    scalar=wti[:, col:col + 1], in1=tgts[y],
                                    op0=Alu.mult, op1=Alu.add)
                        if k != 0:
                            for y in range(BLKROWS):
                                meng = nc.vector if y < DVE_Y else nc.gpsimd
                                meng.tensor_tensor(out=m[:, y, :], in0=m[:, y, :],
                                                   in1=tgts[y], op=Alu.max)

                # ---- tail: A-contraction, transpose, sigmoid, store ----
                if no_tail:
                    continue
                logits = opool.tile([128, G4 * BLKROWS], f32, tag="logits", name="logits")
                prod = opool.tile([128, BLKROWS, C], bf16, tag="prod", name="prod")
                for g in range(G4):
                    TT(out=prod[:],
                       in0=m[:],
                       in1=arep_sb[:, g * C:(g + 1) * C].unsqueeze(1).broadcast_to([128, BLKROWS, C]),
                       op=Alu.mult)
                    nc.vector.tensor_reduce(
                        out=logits[:, g * BLKROWS:(g + 1) * BLKROWS],
                        in_=prod[:], axis=mybir.AxisListType.X, op=Alu.add)
                TT(out=logits, in0=logits, in1=c0x_sb[:], op=Alu.add)
                logb = opool.tile([128, G4 * BLKROWS], bf16, tag="logb", name="logb")
                nc.vector.tensor_copy(out=logb, in_=logits)
                logT = opool.tile([G4 * BLKROWS, 128], bf16, tag="logT", name="logT")
                for bi in range(2):
                    for bj in range(4):
                        nc.vector.transpose(
                            out=logT[32 * bi:32 * (bi + 1), 32 * bj:32 * (bj + 1)],
                            in_=logb[32 * bj:32 * (bj + 1), 32 * bi:32 * (bi + 1)])
                attT = opool.tile([G4 * BLKROWS, 128], f32, tag="attT", name="attT")
                nc.scalar.activation(out=attT[:], in_=logT[:],
                                     func=mybir.ActivationFunctionType.Sigmoid)
                attrep = opool.tile([G4 * BLKROWS, C // G4, 128], f32, tag="attrep", name="attrep", bufs=1)
                nc.vector.tensor_copy(
                    out=attrep[:],
                    in_=attT[:].unsqueeze(1).broadcast_to([G4 * BLKROWS, C // G4, 128]))
                # store: partition p (within g-group) = y16; dst c = g+4j
                for g in range(G4):
                    # dst elem (p=y16, j, x) -> out[c=g+4j, y=blk*16+p, x]
                    dst = bass.AP(tensor=out, offset=g * (ROWS * W) + blk * BLKROWS * W,
                                  ap=[[W, BLKROWS], [G4 * ROWS * W, C // G4], [1, W]])
                    nc.sync.dma_start(
                        out=dst,
                        in_=attrep[g * BLKROWS:(g + 1) * BLKROWS, :, :])

    _split_waits(nc)
    return nc


def _marshal(inputs):
    x = np.ascontiguousarray(inputs["x"], dtype=np.float32)
    offset = np.ascontiguousarray(inputs["offset"], dtype=np.float32)
    W0 = np.asarray(inputs["W0"], np.float32); b0 = np.asarray(inputs["b0"], np.float32)
    gamma = np.asarray(inputs["gamma"], np.float32); beta = np.asarray(inputs["beta"], np.float32)
    rm = np.asarray(inputs["run_mean"], np.float32); rv = np.asarray(inputs["run_var"], np.float32)
    W1 = np.asarray(inputs["W1"], np.float32); b1 = np.asarray(inputs["b1"], np.float32)

    inv = gamma / np.sqrt(rv + BN_EPS)
    A = (W1 * inv[None, :]) @ W0              # (4, 256)
    c0 = W1 @ (inv * (b0 - rm) + beta) + b1   # (4,)

    arep = np.broadcast_to(A.reshape(1, G4 * C).astype(ml_dtypes.bfloat16),
                           (128, G4 * C)).copy()
    c0rep = np.broadcast_to(c0.reshape(1, G4), (128, G4)).astype(np.float32).copy()
    xgrid = np.arange(128, dtype=np.float32).reshape(128, 1).copy()

    ky = np.repeat(np.arange(-1, 2), 3).astype(np.float32)   # k//3 - 1
    kx = np.tile(np.arange(-1, 2), 3).astype(np.float32)     # k%3 - 1

    NT = 16788  # table rows: s in [0, (128+2)*128+129 + pad]
    xf_b = []
    for b in range(B):
        # F' = image rows -2..129 zero-padded, +1 position shift (xb1 = x0+1+1)
        Ff = np.zeros((132 * W + 2 + 130, C), ml_dtypes.bfloat16)
        Ff[2 * W + 1:2 * W + 1 + NPOS] = x[b].transpose(1, 2, 0).reshape(NPOS, C).astype(ml_dtypes.bfloat16)
        T = np.concatenate([Ff[0:NT], Ff[1:NT + 1], Ff[W:NT + W], Ff[W + 1:NT + W + 1]], axis=1)
        xf_b.append(np.ascontiguousarray(T))

    in_maps = []
    for core in range(NCORES):
        b = core // 4
        r0 = (core % 4) * ROWS
        # off_px[x, k, y, c] = offset[b, 2k+c, r0+y, x]
        off = offset[b].reshape(NK, 2, H, W)[:, :, r0:r0 + ROWS, :]
        off_px = off.transpose(3, 0, 2, 1).reshape(128, NK * ROWS * 2).copy()
        yv = np.arange(r0, r0 + ROWS, dtype=np.float32)
        ykc = (yv[None, :] + ky[:, None] - 0.5).reshape(1, NK * ROWS)
        ykc = np.broadcast_to(ykc, (128, NK * ROWS)).astype(np.float32).copy()
        xkc = np.broadcast_to((kx[:, None] - 0.5) * np.ones((1, ROWS), np.float32),
                              (NK, ROWS)).reshape(1, NK * ROWS)
        xkc = np.broadcast_to(xkc, (128, NK * ROWS)).astype(np.float32).copy()
        c0x = np.repeat(c0.reshape(G4, 1), BLKROWS, axis=1).reshape(1, G4 * BLKROWS)
        c0x = np.broadcast_to(c0x, (128, G4 * BLKROWS)).astype(np.float32).copy()
        in_maps.append(dict(xf=xf_b[b], offp=off_px, yk=ykc, xk=xkc,
                            xg=xgrid, arep=arep, c0p=c0rep, c0xp=c0x))
    return in_maps


def kernel(**inputs):
    if "nc" not in _prog_cache:
        _prog_cache["nc"] = _build_program()
    nc = _prog_cache["nc"]
    in_maps = _marshal(inputs)
    res = run_bass_kernel_spmd(nc, in_maps, list(range(NCORES)))
    out = np.zeros((B, C, H, W), np.float32)
    for core in range(NCORES):
        b = core // 4
        r0 = (core % 4) * ROWS
        out[b, :, r0:r0 + ROWS, :] = res.results[core]["out"]
    return out



# revision 8
# speedup vs baseline: 1.3606x; 1.3606x over previous
# Trainium2 Bass kernel for nn_DeformSpaceAttentionv2 (deformable 3x3 max-
# sampling attention). Self-contained: hardcodes all shapes/sharding.
#
# Math: the whole channel pipeline after the deformable-unfold-max collapses
# to logits = A @ feat + c0 with A = W1*diag(gamma/sqrt(var+eps))*W0 (4x256),
# so per pixel we need feat[c] = max_k bilinear_k(x)[c], then a 4-way
# contraction, sigmoid, and channel-tiling (done host-side: pure replication).
#
# Sharding: 8 cores = batch (2) x 32-row bands (4). Per core:
#  - Vector engine computes bilinear corner weights / validity / gather
#    indices (floor via round-to-nearest cast tricks),
#  - GPSIMD issues 288 one-index-per-partition indirect gathers (9 kernel
#    points x 32 rows) from a precomputed 4-corner neighborhood table in HBM
#    (T[s] = x-channels at positions s, s+1, s+128, s+129 of the zero-padded
#    image; 1024 bf16 elems/row). This is the kernel's hard floor: SWDGE
#    descriptor-gen costs ~1us/gather on Pool and indirect DMA is
#    gpsimd-only (multi-index and dma_gather probed broken on this path).
#  - PE does the bilinear corner MAC: per (k,y) the per-pixel corner weight
#    is placed on the diagonal of a 128x128 stationary matrix (built with a
#    single 4x-mode tensor_scalar vs the identity), and 4 accumulating
#    matmuls (one per corner) compute sample = sum_j diag(w_j) @ G_j into
#    PSUM. This moves the whole multiply-add load off DVE (which was the
#    baseline bottleneck at 94% busy).
#  - DVE takes a running max over the 9 samples straight out of PSUM, then
#    contracts with A via fused tensor_tensor_reduce (c0 folded in as the
#    reduce seed), 32x32 transposes; ACT applies sigmoid; stores are [4,16,W]
#    slices - the 64x channel replication happens on host.
import numpy as np
import ml_dtypes

import concourse.bass as bass
import concourse.tile as tile
from concourse import mybir
from concourse.bass_utils import run_bass_kernel_spmd
from concourse.masks import make_identity

BN_EPS = 1e-5
B, C, H, W = 2, 256, 128, 128
G4 = 4
ROWS = 32            # output rows per core
NCORES = 8
NPOS = H * W         # 16384
NK = 9
BLKS = 2             # 16-row blocks per core
BLKROWS = 16

f32 = mybir.dt.float32
bf16 = mybir.dt.bfloat16
i16 = mybir.dt.int16
i32 = mybir.dt.int32

_prog_cache = {}


def _split_waits(nc, max_waits=1):
    """walrus codegen supports only 1 sem-wait per instruction; split extras
    onto preceding NoOps."""
    for bb in nc.m.functions[0].blocks:
        new_insts = []
        for ins in bb.instructions:
            si = ins.sync_info
            if si is not None and si.on_wait and len(si.on_wait) > max_waits:
                waits = list(si.on_wait)
                extra, keep = waits[:-max_waits], waits[-max_waits:]
                for i in range(0, len(extra), max_waits):
                    chunk = extra[i:i + max_waits]
                    nop = mybir.InstNoOp(name=f"{ins.name}-wsplit-{i}", ins=[], outs=[])
                    nop.engine = ins.engine
                    nop.sync_info = mybir.SyncInfo(on_wait=chunk, on_update=[])
                    new_insts.append(nop)
                si.on_wait = keep
            new_insts.append(ins)
        bb.instructions[:] = new_insts


def _build_program():
    nc = bass.Bass("TRN2", target_bir_lowering=False)

    xf = nc.declare_dram_parameter("xf", [16788, 4 * C], bf16, isOutput=False)
    offp = nc.declare_dram_parameter("offp", [128, NK * ROWS * 2], f32, isOutput=False)
    yk = nc.declare_dram_parameter("yk", [128, NK * ROWS], f32, isOutput=False)
    xk = nc.declare_dram_parameter("xk", [128, NK * ROWS], f32, isOutput=False)
    xg = nc.declare_dram_parameter("xg", [128, 1], f32, isOutput=False)
    arep = nc.declare_dram_parameter("arep", [128, G4 * C], bf16, isOutput=False)
    c0xp = nc.declare_dram_parameter("c0xp", [128, G4 * BLKROWS], f32, isOutput=False)
    out = nc.declare_dram_parameter("out", [G4, ROWS, W], f32, isOutput=True)

    NC_ = NK * ROWS          # 288 weight columns
    with tile.TileContext(nc) as tc:
        with (
            tc.tile_pool(name="consts", bufs=1) as consts,
            tc.tile_pool(name="wchain", bufs=1) as wchain,
            tc.tile_pool(name="gpool", bufs=2) as gpool,
            tc.tile_pool(name="dpool", bufs=4) as dpool,
            tc.tile_pool(name="mpool", bufs=2) as mpool,
            tc.tile_pool(name="opool", bufs=2) as opool,
            tc.tile_pool(name="psum", bufs=1, space="PSUM") as psum_pool,
        ):
            # ---- const loads ----
            offp_sb = consts.tile([128, NC_ * 2], f32)
            nc.sync.dma_start(out=offp_sb, in_=offp[:, :])
            yk_sb = consts.tile([128, NC_], f32)
            nc.sync.dma_start(out=yk_sb, in_=yk[:, :])
            xk_sb = consts.tile([128, NC_], f32)
            nc.sync.dma_start(out=xk_sb, in_=xk[:, :])
            xg_sb = consts.tile([128, 1], f32)
            nc.sync.dma_start(out=xg_sb, in_=xg[:, :])
            arep_sb = consts.tile([128, G4 * C], bf16)
            nc.sync.dma_start(out=arep_sb, in_=arep[:, :])
            c0x_sb = consts.tile([128, G4 * BLKROWS], f32)
            nc.sync.dma_start(out=c0x_sb, in_=c0xp[:, :])
            id_sb = consts.tile([128, 128], bf16)
            make_identity(nc, id_sb[:])

            Alu = mybir.AluOpType
            TT = nc.vector.tensor_tensor
            TS = nc.vector.tensor_scalar
            STT = nc.vector.scalar_tensor_tensor

            def wt(name):
                return wchain.tile([128, NC_], f32, tag=name, name=name)

            # ---- weight / index chain (all [128, 288] f32) ----
            off_y = offp_sb[:].rearrange("p (m c) -> p m c", c=2)[:, :, 0]
            off_x = offp_sb[:].rearrange("p (m c) -> p m c", c=2)[:, :, 1]

            ty = wt("ty"); TT(out=ty, in0=off_y, in1=yk_sb[:], op=Alu.add)      # py-0.5
            tx0 = wt("tx0"); TT(out=tx0, in0=off_x, in1=xk_sb[:], op=Alu.add)
            tx = wt("tx"); TS(out=tx, in0=tx0, scalar1=xg_sb[:, 0:1], scalar2=None, op0=Alu.add)  # px-0.5

            y0i = wchain.tile([128, NC_], i32, tag="y0i", name="y0i")
            nc.vector.tensor_copy(out=y0i, in_=ty)                             # rne(py-0.5)=floor(py)
            y0f = wt("y0f"); nc.vector.tensor_copy(out=y0f, in_=y0i)
            fy = wt("fy"); STT(out=fy, in0=ty, scalar=0.5, in1=y0f, op0=Alu.add, op1=Alu.subtract)

            x0i = wchain.tile([128, NC_], i32, tag="x0i", name="x0i")
            nc.vector.tensor_copy(out=x0i, in_=tx)
            x0f = wt("x0f"); nc.vector.tensor_copy(out=x0f, in_=x0i)
            fx = wt("fx"); STT(out=fx, in0=tx, scalar=0.5, in1=x0f, op0=Alu.add, op1=Alu.subtract)

            y0c = wt("y0c"); TS(out=y0c, in0=y0f, scalar1=0.0, scalar2=127.0, op0=Alu.max, op1=Alu.min)
            v0 = wt("v0"); TT(out=v0, in0=y0f, in1=y0c, op=Alu.is_equal)
            y1f = wt("y1f"); TS(out=y1f, in0=y0f, scalar1=1.0, scalar2=None, op0=Alu.add)
            y1c = wt("y1c"); TS(out=y1c, in0=y1f, scalar1=0.0, scalar2=127.0, op0=Alu.max, op1=Alu.min)
            v1 = wt("v1"); TT(out=v1, in0=y1f, in1=y1c, op=Alu.is_equal)

            xc0 = wt("xc0"); TS(out=xc0, in0=x0f, scalar1=0.0, scalar2=127.0, op0=Alu.max, op1=Alu.min)
            vx0 = wt("vx0"); TT(out=vx0, in0=x0f, in1=xc0, op=Alu.is_equal)
            x1f = wt("x1f"); TS(out=x1f, in0=x0f, scalar1=1.0, scalar2=None, op0=Alu.add)
            xc1 = wt("xc1"); TS(out=xc1, in0=x1f, scalar1=0.0, scalar2=127.0, op0=Alu.max, op1=Alu.min)
            vx1 = wt("vx1"); TT(out=vx1, in0=x1f, in1=xc1, op=Alu.is_equal)

            wy0 = wt("wy0"); TS(out=wy0, in0=fy, scalar1=-1.0, scalar2=1.0, op0=Alu.mult, op1=Alu.add)
            wy0v = wt("wy0v"); TT(out=wy0v, in0=wy0, in1=v0, op=Alu.mult)
            wy1v = wt("wy1v"); TT(out=wy1v, in0=fy, in1=v1, op=Alu.mult)
            wx0 = wt("wx0"); TS(out=wx0, in0=fx, scalar1=-1.0, scalar2=1.0, op0=Alu.mult, op1=Alu.add)
            wx0v = wt("wx0v"); TT(out=wx0v, in0=wx0, in1=vx0, op=Alu.mult)
            wx1v = wt("wx1v"); TT(out=wx1v, in0=fx, in1=vx1, op=Alu.mult)

            w00 = wt("w00"); TT(out=w00, in0=wy0v, in1=wx0v, op=Alu.mult)
            w01 = wt("w01"); TT(out=w01, in0=wy0v, in1=wx1v, op=Alu.mult)
            w10 = wt("w10"); TT(out=w10, in0=wy1v, in1=wx0v, op=Alu.mult)
            w11 = wt("w11"); TT(out=w11, in0=wy1v, in1=wx1v, op=Alu.mult)
            wq = (w00, w01, w10, w11)

            xb = wt("xb"); TS(out=xb, in0=x0f, scalar1=-1.0, scalar2=128.0, op0=Alu.max, op1=Alu.min)
            xb1 = wt("xb1"); TS(out=xb1, in0=xb, scalar1=1.0, scalar2=None, op0=Alu.add)
            y0s = wt("y0s"); TS(out=y0s, in0=y0f, scalar1=-2.0, scalar2=128.0, op0=Alu.max, op1=Alu.min)
            xb2 = wt("xb2"); TS(out=xb2, in0=xb1, scalar1=256.0, scalar2=None, op0=Alu.add)
            sf = wt("sf"); STT(out=sf, in0=y0s, scalar=128.0, in1=xb2, op0=Alu.mult, op1=Alu.add)
            sidx = wchain.tile([128, NC_], i32, tag="sidx", name="sidx")
            nc.vector.tensor_copy(out=sidx, in_=sf)

            # ---- main loop ----
            for blk in range(BLKS):
                m = mpool.tile([128, BLKROWS, C], bf16, tag="m", name="m")
                for k in range(NK):
                    Gq = gpool.tile([128, BLKROWS, 4 * C], bf16, tag="Gq", name="Gq")
                    for y in range(BLKROWS):
                        col = k * ROWS + blk * BLKROWS + y
                        nc.gpsimd.indirect_dma_start(
                            out=Gq[:, y, :], out_offset=None, in_=xf[:, :],
                            in_offset=bass.IndirectOffsetOnAxis(
                                ap=sidx[:, col:col + 1], axis=0))
                    for y in range(BLKROWS):
                        col = k * ROWS + blk * BLKROWS + y
                        dq = dpool.tile([128, 4, 128], bf16, tag=f"dq{y % 4}",
                                        name=f"dq{y % 4}")
                        for j in range(4):
                            TS(out=dq[:, j, :], in0=id_sb[:],
                               scalar1=wq[j][:, col:col + 1], scalar2=None,
                               op0=Alu.mult)
                        ps = psum_pool.tile([128, C], f32, tag=f"ps{y % 4}",
                                            name=f"ps{y % 4}")
                        for j in range(4):
                            nc.tensor.matmul(
                                out=ps[:, :], lhsT=dq[:, j, :],
                                rhs=Gq[:, y, j * C:(j + 1) * C],
                                start=(j == 0), stop=(j == 3))
                        if k == 0:
                            nc.vector.tensor_copy(out=m[:, y, :], in_=ps[:, :])
                        else:
                            TT(out=m[:, y, :], in0=m[:, y, :], in1=ps[:, :],
                               op=Alu.max)

                # ---- tail: A-contraction (STT mult with sum-accumulator),
                #      +c0, transpose, sigmoid, store [4,16,W] slices ----
                logits = opool.tile([128, G4 * BLKROWS], f32, tag="logits", name="logits")
                for g in range(G4):
                    for y in range(BLKROWS):
                        dump = opool.tile([128, C], bf16, tag=f"dump{y % 4}",
                                          name=f"dump{y % 4}")
                        nc.vector.scalar_tensor_tensor(
                            out=dump[:, :], in0=m[:, y, :], scalar=1.0,
                            in1=arep_sb[:, g * C:(g + 1) * C],
                            op0=Alu.mult, op1=Alu.mult,
                            accum_out=logits[:, g * BLKROWS + y:g * BLKROWS + y + 1])
                logits2 = opool.tile([128, G4 * BLKROWS], f32, tag="logits2", name="logits2")
                TT(out=logits2, in0=logits, in1=c0x_sb[:], op=Alu.add)
                logb = opool.tile([128, G4 * BLKROWS], bf16, tag="logb", name="logb")
                nc.vector.tensor_copy(out=logb, in_=logits2)
                logT = opool.tile([G4 * BLKROWS, 128], bf16, tag="logT", name="logT")
                for bi in range(2):
                    for bj in range(4):
                        nc.vector.transpose(
                            out=logT[32 * bi:32 * (bi + 1), 32 * bj:32 * (bj + 1)],
                            in_=logb[32 * bj:32 * (bj + 1), 32 * bi:32 * (bi + 1)])
                att = opool.tile([G4 * BLKROWS, 128], f32, tag="att", name="att")
                nc.scalar.activation(out=att[:], in_=logT[:],
                                     func=mybir.ActivationFunctionType.Sigmoid)
                for g in range(G4):
                    dst = bass.AP(tensor=out, offset=g * (ROWS * W) + blk * BLKROWS * W,
                                  ap=[[W, BLKROWS], [1, W]])
                    nc.sync.dma_start(
                        out=dst, in_=att[g * BLKROWS:(g + 1) * BLKROWS, :])

    _split_waits(nc)
    return nc


def _marshal(inputs):
    x = np.ascontiguousarray(inputs["x"], dtype=np.float32)
    offset = np.ascontiguousarray(inputs["offset"], dtype=np.float32)
    W0 = np.asarray(inputs["W0"], np.float32); b0 = np.asarray(inputs["b0"], np.float32)
    gamma = np.asarray(inputs["gamma"], np.float32); beta = np.asarray(inputs["beta"], np.float32)
    rm = np.asarray(inputs["run_mean"], np.float32); rv = np.asarray(inputs["run_var"], np.float32)
    W1 = np.asarray(inputs["W1"], np.float32); b1 = np.asarray(inputs["b1"], np.float32)

    inv = gamma / np.sqrt(rv + BN_EPS)
    A = (W1 * inv[None, :]) @ W0              # (4, 256)
    c0 = W1 @ (inv * (b0 - rm) + beta) + b1   # (4,)

    arep = np.broadcast_to(A.reshape(1, G4 * C).astype(ml_dtypes.bfloat16),
                           (128, G4 * C)).copy()
    c0x = np.repeat(c0.reshape(G4, 1), BLKROWS, axis=1).reshape(1, G4 * BLKROWS)
    c0x = np.broadcast_to(c0x, (128, G4 * BLKROWS)).astype(np.float32).copy()
    xgrid = np.arange(128, dtype=np.float32).reshape(128, 1).copy()

    ky = np.repeat(np.arange(-1, 2), 3).astype(np.float32)   # k//3 - 1
    kx = np.tile(np.arange(-1, 2), 3).astype(np.float32)     # k%3 - 1

    NT = 16788  # table rows: s in [0, (128+2)*128+129 + pad]
    xf_b = []
    for b in range(B):
        # F' = image rows -2..129 zero-padded, +1 position shift (xb1 = x0+1+1)
        Ff = np.zeros((132 * W + 2 + 130, C), ml_dtypes.bfloat16)
        Ff[2 * W + 1:2 * W + 1 + NPOS] = x[b].transpose(1, 2, 0).reshape(NPOS, C).astype(ml_dtypes.bfloat16)
        T = np.concatenate([Ff[0:NT], Ff[1:NT + 1], Ff[W:NT + W], Ff[W + 1:NT + W + 1]], axis=1)
        xf_b.append(np.ascontiguousarray(T))

    in_maps = []
    for core in range(NCORES):
        b = core // 4
        r0 = (core % 4) * ROWS
        # off_px[x, k, y, c] = offset[b, 2k+c, r0+y, x]
        off = offset[b].reshape(NK, 2, H, W)[:, :, r0:r0 + ROWS, :]
        off_px = off.transpose(3, 0, 2, 1).reshape(128, NK * ROWS * 2).copy()
        yv = np.arange(r0, r0 + ROWS, dtype=np.float32)
        ykc = (yv[None, :] + ky[:, None] - 0.5).reshape(1, NK * ROWS)
        ykc = np.broadcast_to(ykc, (128, NK * ROWS)).astype(np.float32).copy()
        xkc = np.broadcast_to((kx[:, None] - 0.5) * np.ones((1, ROWS), np.float32),
                              (NK, ROWS)).reshape(1, NK * ROWS)
        xkc = np.broadcast_to(xkc, (128, NK * ROWS)).astype(np.float32).copy()
        in_maps.append(dict(xf=xf_b[b], offp=off_px, yk=ykc, xk=xkc,
                            xg=xgrid, arep=arep, c0xp=c0x))
    return in_maps


def kernel(**inputs):
    if "nc" not in _prog_cache:
        _prog_cache["nc"] = _build_program()
    nc = _prog_cache["nc"]
    in_maps = _marshal(inputs)
    res = run_bass_kernel_spmd(nc, in_maps, list(range(NCORES)))
    out = np.zeros((B, C, H, W), np.float32)
    for core in range(NCORES):
        b = core // 4
        r0 = (core % 4) * ROWS
        att = res.results[core]["out"]                      # (4, 32, 128)
        out[b, :, r0:r0 + ROWS, :] = np.tile(att, (C // G4, 1, 1))
    return out


# revision 12
# speedup vs baseline: 1.4024x; 1.0307x over previous
# Trainium2 Bass kernel for nn_DeformSpaceAttentionv2 (deformable 3x3 max-
# sampling attention). Self-contained: hardcodes all shapes/sharding.
#
# Math: the whole channel pipeline after the deformable-unfold-max collapses
# to logits = A @ feat + c0 with A = W1*diag(gamma/sqrt(var+eps))*W0 (4x256),
# so per pixel we need feat[c] = max_k bilinear_k(x)[c], then a 4-way
# contraction, sigmoid, and channel-tiling (done host-side: pure replication).
#
# Sharding: 8 cores = batch (2) x 32-row bands (4). Per core:
#  - Vector engine computes bilinear corner weights / validity / gather
#    indices (floor via round-to-nearest cast tricks),
#  - GPSIMD issues 288 one-index-per-partition indirect gathers (9 kernel
#    points x 32 rows) from a precomputed 4-corner neighborhood table in HBM
#    (T[s] = x-channels at positions s, s+1, s+128, s+129 of the zero-padded
#    image; 1024 bf16 elems/row). This is the kernel's hard floor: SWDGE
#    descriptor-gen costs ~1us/gather on Pool and indirect DMA is
#    gpsimd-only (multi-index and dma_gather probed broken on this path).
#  - PE does the bilinear corner MAC: per (k,y) the per-pixel corner weight
#    is placed on the diagonal of a 128x128 stationary matrix (built with a
#    single 4x-mode tensor_scalar vs the identity), and 4 accumulating
#    matmuls (one per corner) compute sample = sum_j diag(w_j) @ G_j into
#    PSUM. This moves the whole multiply-add load off DVE (which was the
#    baseline bottleneck at 94% busy).
#  - DVE takes a running max over the 9 samples straight out of PSUM, then
#    contracts with A via fused tensor_tensor_reduce (c0 folded in as the
#    reduce seed), 32x32 transposes; ACT applies sigmoid; stores are [4,16,W]
#    slices - the 64x channel replication happens on host.
import numpy as np
import ml_dtypes

import concourse.bass as bass
import concourse.tile as tile
from concourse import mybir
from concourse.bass_utils import run_bass_kernel_spmd
from concourse.masks import make_identity

BN_EPS = 1e-5
B, C, H, W = 2, 256, 128, 128
G4 = 4
ROWS = 32            # output rows per core
NCORES = 8
NPOS = H * W         # 16384
NK = 9
BLKS = 2             # 16-row blocks per core
BLKROWS = 16

f32 = mybir.dt.float32
bf16 = mybir.dt.bfloat16
i16 = mybir.dt.int16
i32 = mybir.dt.int32

_prog_cache = {}


def _split_waits(nc, max_waits=1):
    """walrus codegen supports only 1 sem-wait per instruction; split extras
    onto preceding NoOps."""
    for bb in nc.m.functions[0].blocks:
        new_insts = []
        for ins in bb.instructions:
            si = ins.sync_info
            if si is not None and si.on_wait and len(si.on_wait) > max_waits:
                waits = list(si.on_wait)
                extra, keep = waits[:-max_waits], waits[-max_waits:]
                for i in range(0, len(extra), max_waits):
                    chunk = extra[i:i + max_waits]
                    nop = mybir.InstNoOp(name=f"{ins.name}-wsplit-{i}", ins=[], outs=[])
                    nop.engine = ins.engine
                    nop.sync_info = mybir.SyncInfo(on_wait=chunk, on_update=[])
                    new_insts.append(nop)
                si.on_wait = keep
            new_insts.append(ins)
        bb.instructions[:] = new_insts


def _build_program():
    nc = bass.Bass("TRN2", target_bir_lowering=False)

    xf = nc.declare_dram_parameter("xf", [16788, 4 * C], bf16, isOutput=False)
    offp = nc.declare_dram_parameter("offp", [128, NK * ROWS * 2], f32, isOutput=False)
    yk = nc.declare_dram_parameter("yk", [128, NK * ROWS], f32, isOutput=False)
    xk = nc.declare_dram_parameter("xk", [128, NK * ROWS], f32, isOutput=False)
    xg = nc.declare_dram_parameter("xg", [128, 1], f32, isOutput=False)
    arep = nc.declare_dram_parameter("arep", [128, G4 * C], bf16, isOutput=False)
    c0xp = nc.declare_dram_parameter("c0xp", [128, G4 * BLKROWS], f32, isOutput=False)
    out = nc.declare_dram_parameter("out", [G4, ROWS, W], f32, isOutput=True)

    NC_ = NK * ROWS          # 288 weight columns
    with tile.TileContext(nc) as tc:
        with (
            tc.tile_pool(name="consts", bufs=1) as consts,
            tc.tile_pool(name="wchain", bufs=1) as wchain,
            tc.tile_pool(name="gpool", bufs=2) as gpool,
            tc.tile_pool(name="dpool", bufs=4) as dpool,
            tc.tile_pool(name="mpool", bufs=2) as mpool,
            tc.tile_pool(name="spool", bufs=3) as spool,
            tc.tile_pool(name="opool", bufs=2) as opool,
            tc.tile_pool(name="psum", bufs=1, space="PSUM") as psum_pool,
        ):
            # ---- const loads ----
            offp_sb = consts.tile([128, NC_ * 2], f32)
            nc.sync.dma_start(out=offp_sb, in_=offp[:, :])
            yk_sb = consts.tile([128, NC_], f32)
            nc.sync.dma_start(out=yk_sb, in_=yk[:, :])
            xk_sb = consts.tile([128, NC_], f32)
            nc.sync.dma_start(out=xk_sb, in_=xk[:, :])
            xg_sb = consts.tile([128, 1], f32)
            nc.sync.dma_start(out=xg_sb, in_=xg[:, :])
            arep_sb = consts.tile([128, G4 * C], bf16)
            nc.sync.dma_start(out=arep_sb, in_=arep[:, :])
            c0x_sb = consts.tile([128, G4 * BLKROWS], f32)
            nc.sync.dma_start(out=c0x_sb, in_=c0xp[:, :])
            id_sb = consts.tile([128, 128], bf16)
            make_identity(nc, id_sb[:])

            Alu = mybir.AluOpType
            TT = nc.vector.tensor_tensor
            TS = nc.vector.tensor_scalar
            STT = nc.vector.scalar_tensor_tensor

            def wt(name):
                return wchain.tile([128, NC_], f32, tag=name, name=name)

            # ---- weight / index chain (all [128, 288] f32) ----
            off_y = offp_sb[:].rearrange("p (m c) -> p m c", c=2)[:, :, 0]
            off_x = offp_sb[:].rearrange("p (m c) -> p m c", c=2)[:, :, 1]

            ty = wt("ty"); TT(out=ty, in0=off_y, in1=yk_sb[:], op=Alu.add)      # py-0.5
            tx0 = wt("tx0"); TT(out=tx0, in0=off_x, in1=xk_sb[:], op=Alu.add)
            tx = wt("tx"); TS(out=tx, in0=tx0, scalar1=xg_sb[:, 0:1], scalar2=None, op0=Alu.add)  # px-0.5

            y0i = wchain.tile([128, NC_], i32, tag="y0i", name="y0i")
            nc.vector.tensor_copy(out=y0i, in_=ty)                             # rne(py-0.5)=floor(py)
            y0f = wt("y0f"); nc.vector.tensor_copy(out=y0f, in_=y0i)
            fy = wt("fy"); STT(out=fy, in0=ty, scalar=0.5, in1=y0f, op0=Alu.add, op1=Alu.subtract)

            x0i = wchain.tile([128, NC_], i32, tag="x0i", name="x0i")
            nc.vector.tensor_copy(out=x0i, in_=tx)
            x0f = wt("x0f"); nc.vector.tensor_copy(out=x0f, in_=x0i)
            fx = wt("fx"); STT(out=fx, in0=tx, scalar=0.5, in1=x0f, op0=Alu.add, op1=Alu.subtract)

            # index chain first so Pool can start gathering ASAP
            xb = wt("xb"); TS(out=xb, in0=x0f, scalar1=-1.0, scalar2=128.0, op0=Alu.max, op1=Alu.min)
            xb1 = wt("xb1"); TS(out=xb1, in0=xb, scalar1=1.0, scalar2=None, op0=Alu.add)
            y0s = wt("y0s"); TS(out=y0s, in0=y0f, scalar1=-2.0, scalar2=128.0, op0=Alu.max, op1=Alu.min)
            xb2 = wt("xb2"); TS(out=xb2, in0=xb1, scalar1=256.0, scalar2=None, op0=Alu.add)
            sf = wt("sf"); STT(out=sf, in0=y0s, scalar=128.0, in1=xb2, op0=Alu.mult, op1=Alu.add)
            sidx = wchain.tile([128, NC_], i32, tag="sidx", name="sidx")
            nc.vector.tensor_copy(out=sidx, in_=sf)

            y0c = wt("y0c"); TS(out=y0c, in0=y0f, scalar1=0.0, scalar2=127.0, op0=Alu.max, op1=Alu.min)
            v0 = wt("v0"); TT(out=v0, in0=y0f, in1=y0c, op=Alu.is_equal)
            y1f = wt("y1f"); TS(out=y1f, in0=y0f, scalar1=1.0, scalar2=None, op0=Alu.add)
            y1c = wt("y1c"); TS(out=y1c, in0=y1f, scalar1=0.0, scalar2=127.0, op0=Alu.max, op1=Alu.min)
            v1 = wt("v1"); TT(out=v1, in0=y1f, in1=y1c, op=Alu.is_equal)

            xc0 = wt("xc0"); TS(out=xc0, in0=x0f, scalar1=0.0, scalar2=127.0, op0=Alu.max, op1=Alu.min)
            vx0 = wt("vx0"); TT(out=vx0, in0=x0f, in1=xc0, op=Alu.is_equal)
            x1f = wt("x1f"); TS(out=x1f, in0=x0f, scalar1=1.0, scalar2=None, op0=Alu.add)
            xc1 = wt("xc1"); TS(out=xc1, in0=x1f, scalar1=0.0, scalar2=127.0, op0=Alu.max, op1=Alu.min)
            vx1 = wt("vx1"); TT(out=vx1, in0=x1f, in1=xc1, op=Alu.is_equal)

            wy0 = wt("wy0"); TS(out=wy0, in0=fy, scalar1=-1.0, scalar2=1.0, op0=Alu.mult, op1=Alu.add)
            wy0v = wt("wy0v"); TT(out=wy0v, in0=wy0, in1=v0, op=Alu.mult)
            wy1v = wt("wy1v"); TT(out=wy1v, in0=fy, in1=v1, op=Alu.mult)
            wx0 = wt("wx0"); TS(out=wx0, in0=fx, scalar1=-1.0, scalar2=1.0, op0=Alu.mult, op1=Alu.add)
            wx0v = wt("wx0v"); TT(out=wx0v, in0=wx0, in1=vx0, op=Alu.mult)
            wx1v = wt("wx1v"); TT(out=wx1v, in0=fx, in1=vx1, op=Alu.mult)

            w00 = wt("w00"); TT(out=w00, in0=wy0v, in1=wx0v, op=Alu.mult)
            w01 = wt("w01"); TT(out=w01, in0=wy0v, in1=wx1v, op=Alu.mult)
            w10 = wt("w10"); TT(out=w10, in0=wy1v, in1=wx0v, op=Alu.mult)
            w11 = wt("w11"); TT(out=w11, in0=wy1v, in1=wx1v, op=Alu.mult)
            wq = (w00, w01, w10, w11)

            # ---- main loop ----
            for blk in range(BLKS):
                m = mpool.tile([128, BLKROWS, C], bf16, tag="m", name="m")
                for k in range(NK):
                    Gq = gpool.tile([128, BLKROWS, 4 * C], bf16, tag="Gq", name="Gq")
                    for y in range(BLKROWS):
                        col = k * ROWS + blk * BLKROWS + y
                        nc.gpsimd.indirect_dma_start(
                            out=Gq[:, y, :], out_offset=None, in_=xf[:, :],
                            in_offset=bass.IndirectOffsetOnAxis(
                                ap=sidx[:, col:col + 1], axis=0))
                    for y in range(BLKROWS):
                        col = k * ROWS + blk * BLKROWS + y
                        dq = dpool.tile([128, 4, 128], bf16, tag=f"dq{y % 4}",
                                        name=f"dq{y % 4}")
                        for j in range(4):
                            TS(out=dq[:, j, :], in0=id_sb[:],
                               scalar1=wq[j][:, col:col + 1], scalar2=None,
                               op0=Alu.mult)
                        ps = psum_pool.tile([128, C], f32, tag=f"ps{y % 4}",
                                            name=f"ps{y % 4}")
                        for j in range(4):
                            nc.tensor.matmul(
                                out=ps[:, :], lhsT=dq[:, j, :],
                                rhs=Gq[:, y, j * C:(j + 1) * C],
                                start=(j == 0), stop=(j == 3))
                        # ACT (otherwise idle) evicts PSUM f32 -> SBUF bf16 so
                        # the DVE max runs in the 2x bf16 mode.
                        if k == 0:
                            nc.scalar.activation(
                                out=m[:, y, :], in_=ps[:, :],
                                func=mybir.ActivationFunctionType.Copy)
                        else:
                            es = spool.tile([128, C], bf16, tag=f"es{y % 4}",
                                            name=f"es{y % 4}")
                            nc.scalar.activation(
                                out=es[:, :], in_=ps[:, :],
                                func=mybir.ActivationFunctionType.Copy)
                            TT(out=m[:, y, :], in0=m[:, y, :], in1=es[:, :],
                               op=Alu.max)

                # ---- tail: A-contraction (STT mult with sum-accumulator),
                #      +c0, transpose, sigmoid, store [4,16,W] slices ----
                logits = opool.tile([128, G4 * BLKROWS], f32, tag="logits", name="logits")
                for g in range(G4):
                    for y in range(BLKROWS):
                        dump = opool.tile([128, C], bf16, tag=f"dump{y % 4}",
                                          name=f"dump{y % 4}")
                        nc.vector.scalar_tensor_tensor(
                            out=dump[:, :], in0=m[:, y, :], scalar=1.0,
                            in1=arep_sb[:, g * C:(g + 1) * C],
                            op0=Alu.mult, op1=Alu.mult,
                            accum_out=logits[:, g * BLKROWS + y:g * BLKROWS + y + 1])
                logits2 = opool.tile([128, G4 * BLKROWS], f32, tag="logits2", name="logits2")
                TT(out=logits2, in0=logits, in1=c0x_sb[:], op=Alu.add)
                logb = opool.tile([128, G4 * BLKROWS], bf16, tag="logb", name="logb")
                nc.vector.tensor_copy(out=logb, in_=logits2)
                logT = opool.tile([G4 * BLKROWS, 128], bf16, tag="logT", name="logT")
                for bi in range(2):
                    for bj in range(4):
                        nc.vector.transpose(
                            out=logT[32 * bi:32 * (bi + 1), 32 * bj:32 * (bj + 1)],
                            in_=logb[32 * bj:32 * (bj + 1), 32 * bi:32 * (bi + 1)])
                att = opool.tile([G4 * BLKROWS, 128], f32, tag="att", name="att")
                nc.scalar.activation(out=att[:], in_=logT[:],
                                     func=mybir.ActivationFunctionType.Sigmoid)
                for g in range(G4):
                    dst = bass.AP(tensor=out, offset=g * (ROWS * W) + blk * BLKROWS * W,
                                  ap=[[W, BLKROWS], [1, W]])
                    nc.sync.dma_start(
                        out=dst, in_=att[g * BLKROWS:(g + 1) * BLKROWS, :])

    _split_waits(nc)
    return nc


def _marshal(inputs):
    x = np.ascontiguousarray(inputs["x"], dtype=np.float32)
    offset = np.ascontiguousarray(inputs["offset"], dtype=np.float32)
    W0 = np.asarray(inputs["W0"], np.float32); b0 = np.asarray(inputs["b0"], np.float32)
    gamma = np.asarray(inputs["gamma"], np.float32); beta = np.asarray(inputs["beta"], np.float32)
    rm = np.asarray(inputs["run_mean"], np.float32); rv = np.asarray(inputs["run_var"], np.float32)
    W1 = np.asarray(inputs["W1"], np.float32); b1 = np.asarray(inputs["b1"], np.float32)

    inv = gamma / np.sqrt(rv + BN_EPS)
    A = (W1 * inv[None, :]) @ W0              # (4, 256)
    c0 = W1 @ (inv * (b0 - rm) + beta) + b1   # (4,)

    arep = np.broadcast_to(A.reshape(1, G4 * C).astype(ml_dtypes.bfloat16),
                           (128, G4 * C)).copy()
    c0x = np.repeat(c0.reshape(G4, 1), BLKROWS, axis=1).reshape(1, G4 * BLKROWS)
    c0x = np.broadcast_to(c0x, (128, G4 * BLKROWS)).astype(np.float32).copy()
    xgrid = np.arange(128, dtype=np.float32).reshape(128, 1).copy()

    ky = np.repeat(np.arange(-1, 2), 3).astype(np.float32)   # k//3 - 1
    kx = np.tile(np.arange(-1, 2), 3).astype(np.float32)     # k%3 - 1

    NT = 16788  # table rows: s in [0, (128+2)*128+129 + pad]
    xf_b = []
    for b in range(B):
        # F' = image rows -2..129 zero-padded, +1 position shift (xb1 = x0+1+1)
        Ff = np.zeros((132 * W + 2 + 130, C), ml_dtypes.bfloat16)
        Ff[2 * W + 1:2 * W + 1 + NPOS] = x[b].transpose(1, 2, 0).reshape(NPOS, C).astype(ml_dtypes.bfloat16)
        T = np.concatenate([Ff[0:NT], Ff[1:NT + 1], Ff[W:NT + W], Ff[W + 1:NT + W + 1]], axis=1)
        xf_b.append(np.ascontiguousarray(T))

    in_maps = []
    for core in range(NCORES):
        b = core // 4
        r0 = (core % 4) * ROWS
        # off_px[x, k, y, c] = offset[b, 2k+c, r0+y, x]
        off = offset[b].reshape(NK, 2, H, W)[:, :, r0:r0 + ROWS, :]
        off_px = off.transpose(3, 0, 2, 1).reshape(128, NK * ROWS * 2).copy()
        yv = np.arange(r0, r0 + ROWS, dtype=np.float32)
        ykc = (yv[None, :] + ky[:, None] - 0.5).reshape(1, NK * ROWS)
        ykc = np.broadcast_to(ykc, (128, NK * ROWS)).astype(np.float32).copy()
        xkc = np.broadcast_to((kx[:, None] - 0.5) * np.ones((1, ROWS), np.float32),
                              (NK, ROWS)).reshape(1, NK * ROWS)
        xkc = np.broadcast_to(xkc, (128, NK * ROWS)).astype(np.float32).copy()
        in_maps.append(dict(xf=xf_b[b], offp=off_px, yk=ykc, xk=xkc,
                            xg=xgrid, arep=arep, c0xp=c0x))
    return in_maps


def kernel(**inputs):
    if "nc" not in _prog_cache:
        _prog_cache["nc"] = _build_program()
    nc = _prog_cache["nc"]
    in_maps = _marshal(inputs)
    res = run_bass_kernel_spmd(nc, in_maps, list(range(NCORES)))
    out = np.zeros((B, C, H, W), np.float32)
    for core in range(NCORES):
        b = core // 4
        r0 = (core % 4) * ROWS
        att = res.results[core]["out"]                      # (4, 32, 128)
        out[b, :, r0:r0 + ROWS, :] = np.tile(att, (C // G4, 1, 1))
    return out


# revision 19
# speedup vs baseline: 1.4404x; 1.0271x over previous
# Trainium2 Bass kernel for nn_DeformSpaceAttentionv2 (deformable 3x3 max-
# sampling attention). Self-contained: hardcodes all shapes/sharding.
#
# Math: the whole channel pipeline after the deformable-unfold-max collapses
# to logits = A @ feat + c0 with A = W1*diag(gamma/sqrt(var+eps))*W0 (4x256),
# so per pixel we need feat[c] = max_k bilinear_k(x)[c], then a 4-way
# contraction, sigmoid, and channel-tiling (done host-side: pure replication).
#
# Sharding: 8 cores = batch (2) x 32-row bands (4). Per core:
#  - Vector engine computes bilinear corner weights / validity / gather
#    indices (floor via round-to-nearest cast tricks),
#  - GPSIMD issues 288 one-index-per-partition indirect gathers (9 kernel
#    points x 32 rows) from a precomputed 4-corner neighborhood table in HBM
#    (T[s] = x-channels at positions s, s+1, s+128, s+129 of the zero-padded
#    image; 1024 bf16 elems/row). This is the kernel's hard floor: SWDGE
#    descriptor-gen costs ~1us/gather on Pool and indirect DMA is
#    gpsimd-only (multi-index and dma_gather probed broken on this path).
#  - PE does the bilinear corner MAC: per (k,y) the per-pixel corner weight
#    is placed on the diagonal of a 128x128 stationary matrix (built with a
#    single 4x-mode tensor_scalar vs the identity), and 4 accumulating
#    matmuls (one per corner) compute sample = sum_j diag(w_j) @ G_j into
#    PSUM. This moves the whole multiply-add load off DVE (which was the
#    baseline bottleneck at 94% busy).
#  - DVE takes a running max over the 9 samples straight out of PSUM, then
#    contracts with A via fused tensor_tensor_reduce (c0 folded in as the
#    reduce seed), 32x32 transposes; ACT applies sigmoid; stores are [4,16,W]
#    slices - the 64x channel replication happens on host.
import numpy as np
import ml_dtypes

import concourse.bass as bass
import concourse.tile as tile
from concourse import mybir
from concourse.bass_utils import run_bass_kernel_spmd
from concourse.masks import make_identity

BN_EPS = 1e-5
B, C, H, W = 2, 256, 128, 128
G4 = 4
ROWS = 32            # output rows per core
NCORES = 8
NPOS = H * W         # 16384
NK = 9
BLKS = 2             # 16-row blocks per core
BLKROWS = 16

f32 = mybir.dt.float32
bf16 = mybir.dt.bfloat16
i16 = mybir.dt.int16
i32 = mybir.dt.int32

_prog_cache = {}


def _split_waits(nc, max_waits=1):
    """walrus codegen supports only 1 sem-wait per instruction; split extras
    onto preceding NoOps."""
    for bb in nc.m.functions[0].blocks:
        new_insts = []
        for ins in bb.instructions:
            si = ins.sync_info
            if si is not None and si.on_wait and len(si.on_wait) > max_waits:
                waits = list(si.on_wait)
                extra, keep = waits[:-max_waits], waits[-max_waits:]
                for i in range(0, len(extra), max_waits):
                    chunk = extra[i:i + max_waits]
                    nop = mybir.InstNoOp(name=f"{ins.name}-wsplit-{i}", ins=[], outs=[])
                    nop.engine = ins.engine
                    nop.sync_info = mybir.SyncInfo(on_wait=chunk, on_update=[])
                    new_insts.append(nop)
                si.on_wait = keep
            new_insts.append(ins)
        bb.instructions[:] = new_insts


def _build_program():
    nc = bass.Bass("TRN2", target_bir_lowering=False)

    xf = nc.declare_dram_parameter("xf", [16788, 4 * C], bf16, isOutput=False)
    offp = nc.declare_dram_parameter("offp", [128, NK * ROWS * 2], f32, isOutput=False)
    yk = nc.declare_dram_parameter("yk", [128, NK * ROWS], f32, isOutput=False)
    xk = nc.declare_dram_parameter("xk", [128, NK * ROWS], f32, isOutput=False)
    xg = nc.declare_dram_parameter("xg", [128, 1], f32, isOutput=False)
    arep = nc.declare_dram_parameter("arep", [128, G4 * C], bf16, isOutput=False)
    c0xp = nc.declare_dram_parameter("c0xp", [128, G4 * BLKROWS], f32, isOutput=False)
    idp = nc.declare_dram_parameter("idp", [128, 128], bf16, isOutput=False)
    out = nc.declare_dram_parameter("out", [G4, ROWS, W], f32, isOutput=True)

    NC_ = NK * ROWS          # 288 weight columns
    with tile.TileContext(nc) as tc:
        with (
            tc.tile_pool(name="consts", bufs=1) as consts,
            tc.tile_pool(name="wchain", bufs=1) as wchain,
            tc.tile_pool(name="gpool", bufs=2) as gpool,
            tc.tile_pool(name="dpool", bufs=4) as dpool,
            tc.tile_pool(name="mpool", bufs=2) as mpool,
            tc.tile_pool(name="spool", bufs=3) as spool,
            tc.tile_pool(name="opool", bufs=2) as opool,
            tc.tile_pool(name="psum", bufs=1, space="PSUM") as psum_pool,
        ):
            # ---- const loads ----
            offp_sb = consts.tile([128, NC_ * 2], f32)
            nc.sync.dma_start(out=offp_sb, in_=offp[:, :])
            yk_sb = consts.tile([128, NC_], f32)
            nc.sync.dma_start(out=yk_sb, in_=yk[:, :])
            xk_sb = consts.tile([128, NC_], f32)
            nc.sync.dma_start(out=xk_sb, in_=xk[:, :])
            xg_sb = consts.tile([128, 1], f32)
            nc.sync.dma_start(out=xg_sb, in_=xg[:, :])
            arep_sb = consts.tile([128, G4 * C], bf16)
            nc.sync.dma_start(out=arep_sb, in_=arep[:, :])
            c0x_sb = consts.tile([128, G4 * BLKROWS], f32)
            nc.sync.dma_start(out=c0x_sb, in_=c0xp[:, :])
            id_sb = consts.tile([128, 128], bf16)
            nc.sync.dma_start(out=id_sb, in_=idp[:, :])

            Alu = mybir.AluOpType
            TT = nc.vector.tensor_tensor
            TS = nc.vector.tensor_scalar
            STT = nc.vector.scalar_tensor_tensor

            def wt(name, cols=NC_):
                return wchain.tile([128, cols], f32, tag=name, name=name)

            # ---- weight / index chain ----
            off_y = offp_sb[:].rearrange("p (m c) -> p m c", c=2)[:, :, 0]
            off_x = offp_sb[:].rearrange("p (m c) -> p m c", c=2)[:, :, 1]

            # Index chain in two stages: stage A covers k=0 (cols [0:32]) so
            # the Pool gather stream starts ~4us earlier; stage B the rest.
            sidx_t = wchain.tile([128, NC_], i32, tag="sidx", name="sidx")

            def idx_chain(sl, suff):
                tyc = wt("tyc" + suff, sl.stop - sl.start)
                TT(out=tyc, in0=off_y[:, sl], in1=yk_sb[:, sl], op=Alu.add)
                txc0 = wt("txc0" + suff, sl.stop - sl.start)
                TT(out=txc0, in0=off_x[:, sl], in1=xk_sb[:, sl], op=Alu.add)
                txc = wt("txc" + suff, sl.stop - sl.start)
                TS(out=txc, in0=txc0, scalar1=xg_sb[:, 0:1], scalar2=None, op0=Alu.add)
                yi = wchain.tile([128, sl.stop - sl.start], i32, tag="yi" + suff, name="yi" + suff)
                nc.vector.tensor_copy(out=yi, in_=tyc)       # rne(py-0.5)=floor(py)
                yf = wt("yf" + suff, sl.stop - sl.start)
                nc.vector.tensor_copy(out=yf, in_=yi)
                xi = wchain.tile([128, sl.stop - sl.start], i32, tag="xi" + suff, name="xi" + suff)
                nc.vector.tensor_copy(out=xi, in_=txc)
                xf_ = wt("xf" + suff, sl.stop - sl.start)
                nc.vector.tensor_copy(out=xf_, in_=xi)
                # xb2 = clip(x0, -1, 128) + 257 = clip(x0 + 257, 256, 385)
                xbA = wt("xbA" + suff, sl.stop - sl.start)
                TS(out=xbA, in0=xf_, scalar1=257.0, scalar2=256.0, op0=Alu.add, op1=Alu.max)
                xbB = wt("xbB" + suff, sl.stop - sl.start)
                TS(out=xbB, in0=xbA, scalar1=385.0, scalar2=None, op0=Alu.min)
                y0s = wt("y0s" + suff, sl.stop - sl.start)
                TS(out=y0s, in0=yf, scalar1=-2.0, scalar2=128.0, op0=Alu.max, op1=Alu.min)
                sfc = wt("sfc" + suff, sl.stop - sl.start)
                STT(out=sfc, in0=y0s, scalar=128.0, in1=xbB, op0=Alu.mult, op1=Alu.add)
                nc.vector.tensor_copy(out=sidx_t[:, sl], in_=sfc)
                return tyc, txc, yf, xf_

            KCOLS = ROWS  # 32 cols per kernel point
            tyA, txA, y0fA, x0fA = idx_chain(slice(0, KCOLS), "A")
            tyB, txB, y0fB, x0fB = idx_chain(slice(KCOLS, NC_), "B")
            sidx = sidx_t

            # full-width ty/tx/y0f/x0f for the weight chain
            ty = wt("ty"); tx = wt("tx"); y0f = wt("y0f"); x0f = wt("x0f")
            nc.vector.tensor_copy(out=ty[:, 0:KCOLS], in_=tyA)
            nc.vector.tensor_copy(out=ty[:, KCOLS:NC_], in_=tyB)
            nc.vector.tensor_copy(out=tx[:, 0:KCOLS], in_=txA)
            nc.vector.tensor_copy(out=tx[:, KCOLS:NC_], in_=txB)
            nc.vector.tensor_copy(out=y0f[:, 0:KCOLS], in_=y0fA)
            nc.vector.tensor_copy(out=y0f[:, KCOLS:NC_], in_=y0fB)
            nc.vector.tensor_copy(out=x0f[:, 0:KCOLS], in_=x0fA)
            nc.vector.tensor_copy(out=x0f[:, KCOLS:NC_], in_=x0fB)

            fy = wt("fy"); STT(out=fy, in0=ty, scalar=0.5, in1=y0f, op0=Alu.add, op1=Alu.subtract)
            fx = wt("fx"); STT(out=fx, in0=tx, scalar=0.5, in1=x0f, op0=Alu.add, op1=Alu.subtract)

            y0c = wt("y0c"); TS(out=y0c, in0=y0f, scalar1=0.0, scalar2=127.0, op0=Alu.max, op1=Alu.min)
            v0 = wt("v0"); TT(out=v0, in0=y0f, in1=y0c, op=Alu.is_equal)
            y1f = wt("y1f"); TS(out=y1f, in0=y0f, scalar1=1.0, scalar2=None, op0=Alu.add)
            y1c = wt("y1c"); TS(out=y1c, in0=y1f, scalar1=0.0, scalar2=127.0, op0=Alu.max, op1=Alu.min)
            v1 = wt("v1"); TT(out=v1, in0=y1f, in1=y1c, op=Alu.is_equal)

            xc0 = wt("xc0"); TS(out=xc0, in0=x0f, scalar1=0.0, scalar2=127.0, op0=Alu.max, op1=Alu.min)
            vx0 = wt("vx0"); TT(out=vx0, in0=x0f, in1=xc0, op=Alu.is_equal)
            x1f = wt("x1f"); TS(out=x1f, in0=x0f, scalar1=1.0, scalar2=None, op0=Alu.add)
            xc1 = wt("xc1"); TS(out=xc1, in0=x1f, scalar1=0.0, scalar2=127.0, op0=Alu.max, op1=Alu.min)
            vx1 = wt("vx1"); TT(out=vx1, in0=x1f, in1=xc1, op=Alu.is_equal)

            wy0 = wt("wy0"); TS(out=wy0, in0=fy, scalar1=-1.0, scalar2=1.0, op0=Alu.mult, op1=Alu.add)
            wy0v = wt("wy0v"); TT(out=wy0v, in0=wy0, in1=v0, op=Alu.mult)
            wy1v = wt("wy1v"); TT(out=wy1v, in0=fy, in1=v1, op=Alu.mult)
            wx0 = wt("wx0"); TS(out=wx0, in0=fx, scalar1=-1.0, scalar2=1.0, op0=Alu.mult, op1=Alu.add)
            wx0v = wt("wx0v"); TT(out=wx0v, in0=wx0, in1=vx0, op=Alu.mult)
            wx1v = wt("wx1v"); TT(out=wx1v, in0=fx, in1=vx1, op=Alu.mult)

            w00 = wt("w00"); TT(out=w00, in0=wy0v, in1=wx0v, op=Alu.mult)
            w01 = wt("w01"); TT(out=w01, in0=wy0v, in1=wx1v, op=Alu.mult)
            w10 = wt("w10"); TT(out=w10, in0=wy1v, in1=wx0v, op=Alu.mult)
            w11 = wt("w11"); TT(out=w11, in0=wy1v, in1=wx1v, op=Alu.mult)
            wq = (w00, w01, w10, w11)

            # ---- main loop ----
            for blk in range(BLKS):
                m = mpool.tile([128, BLKROWS, C], bf16, tag="m", name="m")
                logits = opool.tile([128, G4 * BLKROWS], f32, tag="logits", name="logits")
                for k in range(NK):
                    Gq = gpool.tile([128, BLKROWS, 4 * C], bf16, tag="Gq", name="Gq")
                    for y in range(BLKROWS):
                        col = k * ROWS + blk * BLKROWS + y
                        nc.gpsimd.indirect_dma_start(
                            out=Gq[:, y, :], out_offset=None, in_=xf[:, :],
                            in_offset=bass.IndirectOffsetOnAxis(
                                ap=sidx[:, col:col + 1], axis=0))
                    for y in range(BLKROWS):
                        col = k * ROWS + blk * BLKROWS + y
                        dq = dpool.tile([128, 4, 128], bf16, tag=f"dq{y % 4}",
                                        name=f"dq{y % 4}")
                        for j in range(4):
                            TS(out=dq[:, j, :], in0=id_sb[:],
                               scalar1=wq[j][:, col:col + 1], scalar2=None,
                               op0=Alu.mult)
                        ps = psum_pool.tile([128, C], f32, tag=f"ps{y % 4}",
                                            name=f"ps{y % 4}")
                        for j in range(4):
                            nc.tensor.matmul(
                                out=ps[:, :], lhsT=dq[:, j, :],
                                rhs=Gq[:, y, j * C:(j + 1) * C],
                                start=(j == 0), stop=(j == 3))
                        # ACT (otherwise idle) evicts PSUM f32 -> SBUF bf16 so
                        # the DVE max runs in the 2x bf16 mode.
                        if k == 0:
                            nc.scalar.activation(
                                out=m[:, y, :], in_=ps[:, :],
                                func=mybir.ActivationFunctionType.Copy)
                        else:
                            es = spool.tile([128, C], bf16, tag=f"es{y % 4}",
                                            name=f"es{y % 4}")
                            nc.scalar.activation(
                                out=es[:, :], in_=ps[:, :],
                                func=mybir.ActivationFunctionType.Copy)
                            TT(out=m[:, y, :], in0=m[:, y, :], in1=es[:, :],
                               op=Alu.max)
                        # interleave the A-contraction with the last kernel
                        # point so only the final row's tail is exposed
                        if k == NK - 1:
                            for g in range(G4):
                                dump = opool.tile([128, C], bf16, tag=f"dump{(4 * y + g) % 4}",
                                                  name=f"dump{(4 * y + g) % 4}")
                                nc.vector.scalar_tensor_tensor(
                                    out=dump[:, :], in0=m[:, y, :], scalar=1.0,
                                    in1=arep_sb[:, g * C:(g + 1) * C],
                                    op0=Alu.mult, op1=Alu.mult,
                                    accum_out=logits[:, g * BLKROWS + y:g * BLKROWS + y + 1])

                # ---- tail: +c0, transpose, sigmoid, store [4,16,W] slices ----
                logits2 = opool.tile([128, G4 * BLKROWS], f32, tag="logits2", name="logits2")
                TT(out=logits2, in0=logits, in1=c0x_sb[:], op=Alu.add)
                logb = opool.tile([128, G4 * BLKROWS], bf16, tag="logb", name="logb")
                nc.vector.tensor_copy(out=logb, in_=logits2)
                logT = opool.tile([G4 * BLKROWS, 128], bf16, tag="logT", name="logT")
                for bi in range(2):
                    for bj in range(4):
                        nc.vector.transpose(
                            out=logT[32 * bi:32 * (bi + 1), 32 * bj:32 * (bj + 1)],
                            in_=logb[32 * bj:32 * (bj + 1), 32 * bi:32 * (bi + 1)])
                att = opool.tile([G4 * BLKROWS, 128], f32, tag="att", name="att")
                nc.scalar.activation(out=att[:], in_=logT[:],
                                     func=mybir.ActivationFunctionType.Sigmoid)
                for g in range(G4):
                    dst = bass.AP(tensor=out, offset=g * (ROWS * W) + blk * BLKROWS * W,
                                  ap=[[W, BLKROWS], [1, W]])
                    nc.sync.dma_start(
                        out=dst, in_=att[g * BLKROWS:(g + 1) * BLKROWS, :])

    _split_waits(nc)
    return nc


def _marshal(inputs):
    x = np.ascontiguousarray(inputs["x"], dtype=np.float32)
    offset = np.ascontiguousarray(inputs["offset"], dtype=np.float32)
    W0 = np.asarray(inputs["W0"], np.float32); b0 = np.asarray(inputs["b0"], np.float32)
    gamma = np.asarray(inputs["gamma"], np.float32); beta = np.asarray(inputs["beta"], np.float32)
    rm = np.asarray(inputs["run_mean"], np.float32); rv = np.asarray(inputs["run_var"], np.float32)
    W1 = np.asarray(inputs["W1"], np.float32); b1 = np.asarray(inputs["b1"], np.float32)

    inv = gamma / np.sqrt(rv + BN_EPS)
    A = (W1 * inv[None, :]) @ W0              # (4, 256)
    c0 = W1 @ (inv * (b0 - rm) + beta) + b1   # (4,)

    arep = np.broadcast_to(A.reshape(1, G4 * C).astype(ml_dtypes.bfloat16),
                           (128, G4 * C)).copy()
    c0x = np.repeat(c0.reshape(G4, 1), BLKROWS, axis=1).reshape(1, G4 * BLKROWS)
    c0x = np.broadcast_to(c0x, (128, G4 * BLKROWS)).astype(np.float32).copy()
    xgrid = np.arange(128, dtype=np.float32).reshape(128, 1).copy()
    idmat = np.eye(128, dtype=ml_dtypes.bfloat16)

    ky = np.repeat(np.arange(-1, 2), 3).astype(np.float32)   # k//3 - 1
    kx = np.tile(np.arange(-1, 2), 3).astype(np.float32)     # k%3 - 1

    NT = 16788  # table rows: s in [0, (128+2)*128+129 + pad]
    xf_b = []
    for b in range(B):
        # F' = image rows -2..129 zero-padded, +1 position shift (xb1 = x0+1+1)
        Ff = np.zeros((132 * W + 2 + 130, C), ml_dtypes.bfloat16)
        Ff[2 * W + 1:2 * W + 1 + NPOS] = x[b].transpose(1, 2, 0).reshape(NPOS, C).astype(ml_dtypes.bfloat16)
        T = np.concatenate([Ff[0:NT], Ff[1:NT + 1], Ff[W:NT + W], Ff[W + 1:NT + W + 1]], axis=1)
        xf_b.append(np.ascontiguousarray(T))

    in_maps = []
    for core in range(NCORES):
        b = core // 4
        r0 = (core % 4) * ROWS
        # off_px[x, k, y, c] = offset[b, 2k+c, r0+y, x]
        off = offset[b].reshape(NK, 2, H, W)[:, :, r0:r0 + ROWS, :]
        off_px = off.transpose(3, 0, 2, 1).reshape(128, NK * ROWS * 2).copy()
        yv = np.arange(r0, r0 + ROWS, dtype=np.float32)
        ykc = (yv[None, :] + ky[:, None] - 0.5).reshape(1, NK * ROWS)
        ykc = np.broadcast_to(ykc, (128, NK * ROWS)).astype(np.float32).copy()
        xkc = np.broadcast_to((kx[:, None] - 0.5) * np.ones((1, ROWS), np.float32),
                              (NK, ROWS)).reshape(1, NK * ROWS)
        xkc = np.broadcast_to(xkc, (128, NK * ROWS)).astype(np.float32).copy()
        in_maps.append(dict(xf=xf_b[b], offp=off_px, yk=ykc, xk=xkc,
                            xg=xgrid, arep=arep, c0xp=c0x, idp=idmat))
    return in_maps


def kernel(**inputs):
    if "nc" not in _prog_cache:
        _prog_cache["nc"] = _build_program()
    nc = _prog_cache["nc"]
    in_maps = _marshal(inputs)
    res = run_bass_kernel_spmd(nc, in_maps, list(range(NCORES)))
    out = np.zeros((B, C, H, W), np.float32)
    for core in range(NCORES):
        b = core // 4
        r0 = (core % 4) * ROWS
        att = res.results[core]["out"]                      # (4, 32, 128)
        out[b, :, r0:r0 + ROWS, :] = np.tile(att, (C // G4, 1, 1))
    return out


# revision 25
# speedup vs baseline: 1.4792x; 1.0269x over previous
# Trainium2 Bass kernel for nn_DeformSpaceAttentionv2 (deformable 3x3 max-
# sampling attention). Self-contained: hardcodes all shapes/sharding.
#
# Math: the whole channel pipeline after the deformable-unfold-max collapses
# to logits = A @ feat + c0 with A = W1*diag(gamma/sqrt(var+eps))*W0 (4x256),
# so per pixel we need feat[c] = max_k bilinear_k(x)[c], then a 4-way
# contraction, sigmoid, and channel-tiling (done host-side: pure replication).
#
# Sharding: 8 cores = batch (2) x 32-row bands (4). Per core:
#  - Vector engine computes bilinear corner weights / validity / gather
#    indices (floor via round-to-nearest cast tricks),
#  - GPSIMD issues 288 one-index-per-partition indirect gathers (9 kernel
#    points x 32 rows) from a precomputed 4-corner neighborhood table in HBM
#    (T[s] = x-channels at positions s, s+1, s+128, s+129 of the zero-padded
#    image; 1024 bf16 elems/row). This is the kernel's hard floor: SWDGE
#    descriptor-gen costs ~1us/gather on Pool and indirect DMA is
#    gpsimd-only (multi-index and dma_gather probed broken on this path).
#  - PE does the bilinear corner MAC: per (k,y) the per-pixel corner weight
#    is placed on the diagonal of a 128x128 stationary matrix (built with a
#    single 4x-mode tensor_scalar vs the identity), and 4 accumulating
#    matmuls (one per corner) compute sample = sum_j diag(w_j) @ G_j into
#    PSUM. This moves the whole multiply-add load off DVE (which was the
#    baseline bottleneck at 94% busy).
#  - DVE takes a running max over the 9 samples straight out of PSUM, then
#    contracts with A via fused tensor_tensor_reduce (c0 folded in as the
#    reduce seed), 32x32 transposes; ACT applies sigmoid; stores are [4,16,W]
#    slices - the 64x channel replication happens on host.
import numpy as np
import ml_dtypes

import concourse.bass as bass
import concourse.tile as tile
from concourse import mybir
from concourse.bass_utils import run_bass_kernel_spmd
from concourse.masks import make_identity

BN_EPS = 1e-5
B, C, H, W = 2, 256, 128, 128
G4 = 4
ROWS = 32            # output rows per core
NCORES = 8
NPOS = H * W         # 16384
NK = 9
BLKS = 2             # 16-row blocks per core
BLKROWS = 16

f32 = mybir.dt.float32
bf16 = mybir.dt.bfloat16
i16 = mybir.dt.int16
i32 = mybir.dt.int32

_prog_cache = {}


def _split_waits(nc, max_waits=1):
    """walrus codegen supports only 1 sem-wait per instruction; split extras
    onto preceding NoOps."""
    for bb in nc.m.functions[0].blocks:
        new_insts = []
        for ins in bb.instructions:
            si = ins.sync_info
            if si is not None and si.on_wait and len(si.on_wait) > max_waits:
                waits = list(si.on_wait)
                extra, keep = waits[:-max_waits], waits[-max_waits:]
                for i in range(0, len(extra), max_waits):
                    chunk = extra[i:i + max_waits]
                    nop = mybir.InstNoOp(name=f"{ins.name}-wsplit-{i}", ins=[], outs=[])
                    nop.engine = ins.engine
                    nop.sync_info = mybir.SyncInfo(on_wait=chunk, on_update=[])
                    new_insts.append(nop)
                si.on_wait = keep
            new_insts.append(ins)
        bb.instructions[:] = new_insts


def _build_program():
    nc = bass.Bass("TRN2", target_bir_lowering=False)

    xf = nc.declare_dram_parameter("xf", [16788, 4 * C], bf16, isOutput=False)
    offp = nc.declare_dram_parameter("offp", [128, NK * ROWS * 2], f32, isOutput=False)
    yk = nc.declare_dram_parameter("yk", [128, NK * ROWS], f32, isOutput=False)
    xk = nc.declare_dram_parameter("xk", [128, NK * ROWS], f32, isOutput=False)
    xg = nc.declare_dram_parameter("xg", [128, 1], f32, isOutput=False)
    atp = nc.declare_dram_parameter("atp", [128, 2 * G4], bf16, isOutput=False)
    c0t = nc.declare_dram_parameter("c0t", [G4, 1], f32, isOutput=False)
    idp = nc.declare_dram_parameter("idp", [128, 128], bf16, isOutput=False)
    out = nc.declare_dram_parameter("out", [G4, ROWS, W], f32, isOutput=True)

    NC_ = NK * ROWS          # 288 weight columns
    with tile.TileContext(nc) as tc:
        with (
            tc.tile_pool(name="consts", bufs=1) as consts,
            tc.tile_pool(name="wchain", bufs=1) as wchain,
            tc.tile_pool(name="gpool", bufs=2) as gpool,
            tc.tile_pool(name="dpool", bufs=4) as dpool,
            tc.tile_pool(name="mpool", bufs=2) as mpool,
            tc.tile_pool(name="spool", bufs=3) as spool,
            tc.tile_pool(name="opool", bufs=2) as opool,
            tc.tile_pool(name="psum", bufs=1, space="PSUM") as psum_pool,
        ):
            # ---- const loads: index-chain inputs first on the SP queue so the
            # first gather launches ASAP; the rest ride the ACT queue ----
            offp_sb = consts.tile([128, NC_ * 2], f32)
            nc.sync.dma_start(out=offp_sb, in_=offp[:, :])
            yk_sb = consts.tile([128, NC_], f32)
            nc.sync.dma_start(out=yk_sb, in_=yk[:, :])
            xk_sb = consts.tile([128, NC_], f32)
            nc.sync.dma_start(out=xk_sb, in_=xk[:, :])
            xg_sb = consts.tile([128, 1], f32)
            nc.sync.dma_start(out=xg_sb, in_=xg[:, :])
            at_sb = consts.tile([128, 2 * G4], bf16)
            nc.scalar.dma_start(out=at_sb, in_=atp[:, :])
            c0_sb = consts.tile([G4, 1], f32)
            nc.scalar.dma_start(out=c0_sb, in_=c0t[:, :])
            id_sb = consts.tile([128, 128], bf16)
            nc.scalar.dma_start(out=id_sb, in_=idp[:, :])

            Alu = mybir.AluOpType
            TT = nc.vector.tensor_tensor
            TS = nc.vector.tensor_scalar
            STT = nc.vector.scalar_tensor_tensor

            def wt(name, cols=NC_):
                return wchain.tile([128, cols], f32, tag=name, name=name)

            # ---- weight / index chain ----
            off_y = offp_sb[:].rearrange("p (m c) -> p m c", c=2)[:, :, 0]
            off_x = offp_sb[:].rearrange("p (m c) -> p m c", c=2)[:, :, 1]

            # Index chain in two stages: stage A covers k=0 (cols [0:32]) so
            # the Pool gather stream starts ~4us earlier; stage B the rest.
            sidx_t = wchain.tile([128, NC_], i32, tag="sidx", name="sidx")

            def idx_chain(sl, suff):
                tyc = wt("tyc" + suff, sl.stop - sl.start)
                TT(out=tyc, in0=off_y[:, sl], in1=yk_sb[:, sl], op=Alu.add)
                txc0 = wt("txc0" + suff, sl.stop - sl.start)
                TT(out=txc0, in0=off_x[:, sl], in1=xk_sb[:, sl], op=Alu.add)
                txc = wt("txc" + suff, sl.stop - sl.start)
                TS(out=txc, in0=txc0, scalar1=xg_sb[:, 0:1], scalar2=None, op0=Alu.add)
                yi = wchain.tile([128, sl.stop - sl.start], i32, tag="yi" + suff, name="yi" + suff)
                nc.vector.tensor_copy(out=yi, in_=tyc)       # rne(py-0.5)=floor(py)
                yf = wt("yf" + suff, sl.stop - sl.start)
                nc.vector.tensor_copy(out=yf, in_=yi)
                xi = wchain.tile([128, sl.stop - sl.start], i32, tag="xi" + suff, name="xi" + suff)
                nc.vector.tensor_copy(out=xi, in_=txc)
                xf_ = wt("xf" + suff, sl.stop - sl.start)
                nc.vector.tensor_copy(out=xf_, in_=xi)
                # xb2 = clip(x0, -1, 128) + 257 = clip(x0 + 257, 256, 385)
                xbA = wt("xbA" + suff, sl.stop - sl.start)
                TS(out=xbA, in0=xf_, scalar1=257.0, scalar2=256.0, op0=Alu.add, op1=Alu.max)
                xbB = wt("xbB" + suff, sl.stop - sl.start)
                TS(out=xbB, in0=xbA, scalar1=385.0, scalar2=None, op0=Alu.min)
                y0s = wt("y0s" + suff, sl.stop - sl.start)
                TS(out=y0s, in0=yf, scalar1=-2.0, scalar2=128.0, op0=Alu.max, op1=Alu.min)
                sfc = wt("sfc" + suff, sl.stop - sl.start)
                STT(out=sfc, in0=y0s, scalar=128.0, in1=xbB, op0=Alu.mult, op1=Alu.add)
                nc.vector.tensor_copy(out=sidx_t[:, sl], in_=sfc)
                return tyc, txc, yf, xf_

            KCOLS = ROWS  # 32 cols per kernel point
            tyA, txA, y0fA, x0fA = idx_chain(slice(0, KCOLS), "A")
            tyB, txB, y0fB, x0fB = idx_chain(slice(KCOLS, NC_), "B")
            sidx = sidx_t

            # full-width ty/tx/y0f/x0f for the weight chain
            ty = wt("ty"); tx = wt("tx"); y0f = wt("y0f"); x0f = wt("x0f")
            nc.vector.tensor_copy(out=ty[:, 0:KCOLS], in_=tyA)
            nc.vector.tensor_copy(out=ty[:, KCOLS:NC_], in_=tyB)
            nc.vector.tensor_copy(out=tx[:, 0:KCOLS], in_=txA)
            nc.vector.tensor_copy(out=tx[:, KCOLS:NC_], in_=txB)
            nc.vector.tensor_copy(out=y0f[:, 0:KCOLS], in_=y0fA)
            nc.vector.tensor_copy(out=y0f[:, KCOLS:NC_], in_=y0fB)
            nc.vector.tensor_copy(out=x0f[:, 0:KCOLS], in_=x0fA)
            nc.vector.tensor_copy(out=x0f[:, KCOLS:NC_], in_=x0fB)

            fy = wt("fy"); STT(out=fy, in0=ty, scalar=0.5, in1=y0f, op0=Alu.add, op1=Alu.subtract)
            fx = wt("fx"); STT(out=fx, in0=tx, scalar=0.5, in1=x0f, op0=Alu.add, op1=Alu.subtract)

            y0c = wt("y0c"); TS(out=y0c, in0=y0f, scalar1=0.0, scalar2=127.0, op0=Alu.max, op1=Alu.min)
            v0 = wt("v0"); TT(out=v0, in0=y0f, in1=y0c, op=Alu.is_equal)
            y1f = wt("y1f"); TS(out=y1f, in0=y0f, scalar1=1.0, scalar2=None, op0=Alu.add)
            y1c = wt("y1c"); TS(out=y1c, in0=y1f, scalar1=0.0, scalar2=127.0, op0=Alu.max, op1=Alu.min)
            v1 = wt("v1"); TT(out=v1, in0=y1f, in1=y1c, op=Alu.is_equal)

            xc0 = wt("xc0"); TS(out=xc0, in0=x0f, scalar1=0.0, scalar2=127.0, op0=Alu.max, op1=Alu.min)
            vx0 = wt("vx0"); TT(out=vx0, in0=x0f, in1=xc0, op=Alu.is_equal)
            x1f = wt("x1f"); TS(out=x1f, in0=x0f, scalar1=1.0, scalar2=None, op0=Alu.add)
            xc1 = wt("xc1"); TS(out=xc1, in0=x1f, scalar1=0.0, scalar2=127.0, op0=Alu.max, op1=Alu.min)
            vx1 = wt("vx1"); TT(out=vx1, in0=x1f, in1=xc1, op=Alu.is_equal)

            wy0 = wt("wy0"); TS(out=wy0, in0=fy, scalar1=-1.0, scalar2=1.0, op0=Alu.mult, op1=Alu.add)
            wy0v = wt("wy0v"); TT(out=wy0v, in0=wy0, in1=v0, op=Alu.mult)
            wy1v = wt("wy1v"); TT(out=wy1v, in0=fy, in1=v1, op=Alu.mult)
            wx0 = wt("wx0"); TS(out=wx0, in0=fx, scalar1=-1.0, scalar2=1.0, op0=Alu.mult, op1=Alu.add)
            wx0v = wt("wx0v"); TT(out=wx0v, in0=wx0, in1=vx0, op=Alu.mult)
            wx1v = wt("wx1v"); TT(out=wx1v, in0=fx, in1=vx1, op=Alu.mult)

            w00 = wt("w00"); TT(out=w00, in0=wy0v, in1=wx0v, op=Alu.mult)
            w01 = wt("w01"); TT(out=w01, in0=wy0v, in1=wx1v, op=Alu.mult)
            w10 = wt("w10"); TT(out=w10, in0=wy1v, in1=wx0v, op=Alu.mult)
            w11 = wt("w11"); TT(out=w11, in0=wy1v, in1=wx1v, op=Alu.mult)
            wq = (w00, w01, w10, w11)

            # ---- main loop ----
            for blk in range(BLKS):
                m = mpool.tile([128, BLKROWS, C], bf16, tag="m", name="m")
                att = opool.tile([G4, BLKROWS, W], f32, tag="att", name="att")
                for k in range(NK):
                    Gq = gpool.tile([128, BLKROWS, 4 * C], bf16, tag="Gq", name="Gq")
                    for y in range(BLKROWS):
                        col = k * ROWS + blk * BLKROWS + y
                        nc.gpsimd.indirect_dma_start(
                            out=Gq[:, y, :], out_offset=None, in_=xf[:, :],
                            in_offset=bass.IndirectOffsetOnAxis(
                                ap=sidx[:, col:col + 1], axis=0))
                    for y in range(BLKROWS):
                        col = k * ROWS + blk * BLKROWS + y
                        dq = dpool.tile([128, 4, 128], bf16, tag=f"dq{y % 4}",
                                        name=f"dq{y % 4}")
                        for j in range(4):
                            TS(out=dq[:, j, :], in0=id_sb[:],
                               scalar1=wq[j][:, col:col + 1], scalar2=None,
                               op0=Alu.mult)
                        ps = psum_pool.tile([128, C], f32, tag=f"ps{y % 4}",
                                            name=f"ps{y % 4}")
                        for j in range(4):
                            nc.tensor.matmul(
                                out=ps[:, :], lhsT=dq[:, j, :],
                                rhs=Gq[:, y, j * C:(j + 1) * C],
                                start=(j == 0), stop=(j == 3))
                        # ACT (otherwise idle) evicts PSUM f32 -> SBUF bf16 so
                        # the DVE max runs in the 2x bf16 mode.
                        if k == 0:
                            nc.scalar.activation(
                                out=m[:, y, :], in_=ps[:, :],
                                func=mybir.ActivationFunctionType.Copy)
                        else:
                            es = spool.tile([128, C], bf16, tag=f"es{y % 4}",
                                            name=f"es{y % 4}")
                            nc.scalar.activation(
                                out=es[:, :], in_=ps[:, :],
                                func=mybir.ActivationFunctionType.Copy)
                            TT(out=m[:, y, :], in0=m[:, y, :], in1=es[:, :],
                               op=Alu.max)
                        # interleave the A-contraction with the last kernel
                        # point so only the final row's tail is exposed:
                        # PE transposes m row -> mT, PE contracts with A^T,
                        # ACT applies sigmoid(logits + c0) straight from PSUM.
                        if k == NK - 1:
                            psT = psum_pool.tile([128, 2, 128], bf16,
                                                 tag=f"psT{y % 2}", name=f"psT{y % 2}")
                            for h in range(2):
                                nc.tensor.transpose(
                                    out=psT[:, h, :],
                                    in_=m[:, y, h * 128:(h + 1) * 128],
                                    identity=id_sb[:])
                            mT = spool.tile([128, 2, 128], bf16, tag=f"mT{y % 2}",
                                            name=f"mT{y % 2}")
                            nc.vector.tensor_copy(out=mT[:], in_=psT[:])
                            psL = psum_pool.tile([G4, 128], f32, tag=f"psL{y % 2}",
                                                 name=f"psL{y % 2}")
                            for h in range(2):
                                nc.tensor.matmul(
                                    out=psL[:, :], lhsT=at_sb[:, h * G4:(h + 1) * G4],
                                    rhs=mT[:, h, :], start=(h == 0), stop=(h == 1))
                            nc.scalar.activation(
                                out=att[:, y, :], in_=psL[:, :],
                                func=mybir.ActivationFunctionType.Sigmoid,
                                bias=c0_sb[:, 0:1])

                # ---- store [4,16,W] block ----
                dst = bass.AP(tensor=out, offset=blk * BLKROWS * W,
                              ap=[[ROWS * W, G4], [W, BLKROWS], [1, W]])
                nc.sync.dma_start(out=dst, in_=att[:, :, :])

    _split_waits(nc)
    return nc


def _marshal(inputs):
    x = np.ascontiguousarray(inputs["x"], dtype=np.float32)
    offset = np.ascontiguousarray(inputs["offset"], dtype=np.float32)
    W0 = np.asarray(inputs["W0"], np.float32); b0 = np.asarray(inputs["b0"], np.float32)
    gamma = np.asarray(inputs["gamma"], np.float32); beta = np.asarray(inputs["beta"], np.float32)
    rm = np.asarray(inputs["run_mean"], np.float32); rv = np.asarray(inputs["run_var"], np.float32)
    W1 = np.asarray(inputs["W1"], np.float32); b1 = np.asarray(inputs["b1"], np.float32)

    inv = gamma / np.sqrt(rv + BN_EPS)
    A = (W1 * inv[None, :]) @ W0              # (4, 256)
    c0 = W1 @ (inv * (b0 - rm) + beta) + b1   # (4,)

    # atp[c', h*4+g] = A[g, h*128+c']  (A^T in two 128-channel chunks)
    atm = A.reshape(G4, 2, 128).transpose(2, 1, 0).reshape(128, 2 * G4)
    atm = np.ascontiguousarray(atm.astype(ml_dtypes.bfloat16))
    c0col = np.ascontiguousarray(c0.reshape(G4, 1).astype(np.float32))
    xgrid = np.arange(128, dtype=np.float32).reshape(128, 1).copy()
    idmat = np.eye(128, dtype=ml_dtypes.bfloat16)

    ky = np.repeat(np.arange(-1, 2), 3).astype(np.float32)   # k//3 - 1
    kx = np.tile(np.arange(-1, 2), 3).astype(np.float32)     # k%3 - 1

    NT = 16788  # table rows: s in [0, (128+2)*128+129 + pad]
    xf_b = []
    for b in range(B):
        # F' = image rows -2..129 zero-padded, +1 position shift (xb1 = x0+1+1)
        Ff = np.zeros((132 * W + 2 + 130, C), ml_dtypes.bfloat16)
        Ff[2 * W + 1:2 * W + 1 + NPOS] = x[b].transpose(1, 2, 0).reshape(NPOS, C).astype(ml_dtypes.bfloat16)
        T = np.concatenate([Ff[0:NT], Ff[1:NT + 1], Ff[W:NT + W], Ff[W + 1:NT + W + 1]], axis=1)
        xf_b.append(np.ascontiguousarray(T))

    in_maps = []
    for core in range(NCORES):
        b = core // 4
        r0 = (core % 4) * ROWS
        # off_px[x, k, y, c] = offset[b, 2k+c, r0+y, x]
        off = offset[b].reshape(NK, 2, H, W)[:, :, r0:r0 + ROWS, :]
        off_px = off.transpose(3, 0, 2, 1).reshape(128, NK * ROWS * 2).copy()
        yv = np.arange(r0, r0 + ROWS, dtype=np.float32)
        ykc = (yv[None, :] + ky[:, None] - 0.5).reshape(1, NK * ROWS)
        ykc = np.broadcast_to(ykc, (128, NK * ROWS)).astype(np.float32).copy()
        xkc = np.broadcast_to((kx[:, None] - 0.5) * np.ones((1, ROWS), np.float32),
                              (NK, ROWS)).reshape(1, NK * ROWS)
        xkc = np.broadcast_to(xkc, (128, NK * ROWS)).astype(np.float32).copy()
        in_maps.append(dict(xf=xf_b[b], offp=off_px, yk=ykc, xk=xkc,
                            xg=xgrid, atp=atm, c0t=c0col, idp=idmat))
    return in_maps


def kernel(**inputs):
    if "nc" not in _prog_cache:
        _prog_cache["nc"] = _build_program()
    nc = _prog_cache["nc"]
    in_maps = _marshal(inputs)
    res = run_bass_kernel_spmd(nc, in_maps, list(range(NCORES)))
    out = np.zeros((B, C, H, W), np.float32)
    for core in range(NCORES):
        b = core // 4
        r0 = (core % 4) * ROWS
        att = res.results[core]["out"]                      # (4, 32, 128)
        out[b, :, r0:r0 + ROWS, :] = np.tile(att, (C // G4, 1, 1))
    return out


# revision 30
# speedup vs baseline: 1.4996x; 1.0138x over previous
# Trainium2 Bass kernel for nn_DeformSpaceAttentionv2 (deformable 3x3 max-
# sampling attention). Self-contained: hardcodes all shapes/sharding.
#
# Math: the whole channel pipeline after the deformable-unfold-max collapses
# to logits = A @ feat + c0 with A = W1*diag(gamma/sqrt(var+eps))*W0 (4x256),
# so per pixel we need feat[c] = max_k bilinear_k(x)[c], then a 4-way
# contraction, sigmoid, and channel-tiling (done host-side: pure replication).
#
# Sharding: 8 cores = batch (2) x 32-row bands (4). Per core:
#  - Vector engine computes bilinear corner weights / validity / gather
#    indices (floor via round-to-nearest cast tricks),
#  - GPSIMD issues 288 one-index-per-partition indirect gathers (9 kernel
#    points x 32 rows) from a precomputed 4-corner neighborhood table in HBM
#    (T[s] = x-channels at positions s, s+1, s+128, s+129 of the zero-padded
#    image; 1024 bf16 elems/row). This is the kernel's hard floor: SWDGE
#    descriptor-gen costs ~1us/gather on Pool and indirect DMA is
#    gpsimd-only (multi-index and dma_gather probed broken on this path).
#  - PE does the bilinear corner MAC: per (k,y) the per-pixel corner weight
#    is placed on the diagonal of a 128x128 stationary matrix (built with a
#    single 4x-mode tensor_scalar vs the identity), and 4 accumulating
#    matmuls (one per corner) compute sample = sum_j diag(w_j) @ G_j into
#    PSUM. This moves the whole multiply-add load off DVE (which was the
#    baseline bottleneck at 94% busy).
#  - DVE takes a running max over the 9 samples straight out of PSUM, then
#    contracts with A via fused tensor_tensor_reduce (c0 folded in as the
#    reduce seed), 32x32 transposes; ACT applies sigmoid; stores are [4,16,W]
#    slices - the 64x channel replication happens on host.
import numpy as np
import ml_dtypes

import concourse.bass as bass
import concourse.tile as tile
from concourse import mybir
from concourse.bass_utils import run_bass_kernel_spmd
from concourse.masks import make_identity

BN_EPS = 1e-5
B, C, H, W = 2, 256, 128, 128
G4 = 4
ROWS = 32            # output rows per core
NCORES = 8
NPOS = H * W         # 16384
NK = 9
BLKS = 2             # 16-row blocks per core
BLKROWS = 16

f32 = mybir.dt.float32
bf16 = mybir.dt.bfloat16
i16 = mybir.dt.int16
i32 = mybir.dt.int32

_prog_cache = {}


def _split_waits(nc, max_waits=1):
    """walrus codegen supports only 1 sem-wait per instruction; split extras
    onto preceding NoOps."""
    for bb in nc.m.functions[0].blocks:
        new_insts = []
        for ins in bb.instructions:
            si = ins.sync_info
            if si is not None and si.on_wait and len(si.on_wait) > max_waits:
                waits = list(si.on_wait)
                extra, keep = waits[:-max_waits], waits[-max_waits:]
                for i in range(0, len(extra), max_waits):
                    chunk = extra[i:i + max_waits]
                    nop = mybir.InstNoOp(name=f"{ins.name}-wsplit-{i}", ins=[], outs=[])
                    nop.engine = ins.engine
                    nop.sync_info = mybir.SyncInfo(on_wait=chunk, on_update=[])
                    new_insts.append(nop)
                si.on_wait = keep
            new_insts.append(ins)
        bb.instructions[:] = new_insts


def _build_program():
    nc = bass.Bass("TRN2", target_bir_lowering=False)

    xf = nc.declare_dram_parameter("xf", [16788, 4 * C], bf16, isOutput=False)
    # fastp = [off k=0 (64) | yk k=0 (32) | xk k=0 (32) | xg (1)] - the minimal
    # inputs for the k=0 index chain, loaded in the first HWDGE slot.
    fastp = nc.declare_dram_parameter("fastp", [128, 129], f32, isOutput=False)
    # bigp = [offp (576) | yk (288) | xk (288)] - everything else, one DMA.
    bigp = nc.declare_dram_parameter("bigp", [128, NK * ROWS * 4 + 1], f32, isOutput=False)
    atp = nc.declare_dram_parameter("atp", [128, 2 * G4], bf16, isOutput=False)
    c0t = nc.declare_dram_parameter("c0t", [G4, 1], f32, isOutput=False)
    idp = nc.declare_dram_parameter("idp", [128, 128], bf16, isOutput=False)
    out = nc.declare_dram_parameter("out", [G4, ROWS, W], f32, isOutput=True)

    NC_ = NK * ROWS          # 288 weight columns
    with tile.TileContext(nc) as tc:
        with (
            tc.tile_pool(name="consts", bufs=1) as consts,
            tc.tile_pool(name="wchain", bufs=1) as wchain,
            tc.tile_pool(name="gpool", bufs=2) as gpool,
            tc.tile_pool(name="dpool", bufs=4) as dpool,
            tc.tile_pool(name="mpool", bufs=2) as mpool,
            tc.tile_pool(name="spool", bufs=3) as spool,
            tc.tile_pool(name="opool", bufs=2) as opool,
            tc.tile_pool(name="psum", bufs=1, space="PSUM") as psum_pool,
        ):
            # ---- const loads: fastp grabs the first HWDGE slot so the k=0
            # index chain (and hence the Pool gather stream) starts ASAP ----
            fast_sb = consts.tile([128, 129], f32)
            nc.sync.dma_start(out=fast_sb, in_=fastp[:, :])
            big_sb = consts.tile([128, NK * ROWS * 4 + 1], f32)
            nc.sync.dma_start(out=big_sb, in_=bigp[:, :])
            at_sb = consts.tile([128, 2 * G4], bf16)
            nc.scalar.dma_start(out=at_sb, in_=atp[:, :])
            c0_sb = consts.tile([G4, 1], f32)
            nc.scalar.dma_start(out=c0_sb, in_=c0t[:, :])
            id_sb = consts.tile([128, 128], bf16)
            nc.scalar.dma_start(out=id_sb, in_=idp[:, :])
            offp_sb = big_sb[:, 0:NC_ * 2]
            yk_sb = big_sb[:, NC_ * 2:NC_ * 3]
            xk_sb = big_sb[:, NC_ * 3:NC_ * 4]
            xg_sb = big_sb[:, NC_ * 4:NC_ * 4 + 1]

            Alu = mybir.AluOpType
            TT = nc.vector.tensor_tensor
            TS = nc.vector.tensor_scalar
            STT = nc.vector.scalar_tensor_tensor

            def wt(name, cols=NC_):
                return wchain.tile([128, cols], f32, tag=name, name=name)

            # ---- weight / index chain ----
            # Index chain in two stages: stage A covers k=0 only, reading the
            # small fastp const (lands ~2us), so gathers start ASAP.
            KCOLS = ROWS  # 32 cols per kernel point

            def idx_chain(n, offy_ap, offx_ap, yk_ap, xk_ap, xg_ap, sidx_tile, suff):
                tyc = wt("tyc" + suff, n)
                TT(out=tyc, in0=offy_ap, in1=yk_ap, op=Alu.add)
                txc0 = wt("txc0" + suff, n)
                TT(out=txc0, in0=offx_ap, in1=xk_ap, op=Alu.add)
                txc = wt("txc" + suff, n)
                TS(out=txc, in0=txc0, scalar1=xg_ap, scalar2=None, op0=Alu.add)
                yi = wchain.tile([128, n], i32, tag="yi" + suff, name="yi" + suff)
                nc.vector.tensor_copy(out=yi, in_=tyc)       # rne(py-0.5)=floor(py)
                yf = wt("yf" + suff, n)
                nc.vector.tensor_copy(out=yf, in_=yi)
                xi = wchain.tile([128, n], i32, tag="xi" + suff, name="xi" + suff)
                nc.vector.tensor_copy(out=xi, in_=txc)
                xf_ = wt("xf" + suff, n)
                nc.vector.tensor_copy(out=xf_, in_=xi)
                # xb2 = clip(x0, -1, 128) + 257 = clip(x0 + 257, 256, 385)
                xbA = wt("xbA" + suff, n)
                TS(out=xbA, in0=xf_, scalar1=257.0, scalar2=256.0, op0=Alu.add, op1=Alu.max)
                xbB = wt("xbB" + suff, n)
                TS(out=xbB, in0=xbA, scalar1=385.0, scalar2=None, op0=Alu.min)
                y0s = wt("y0s" + suff, n)
                TS(out=y0s, in0=yf, scalar1=-2.0, scalar2=128.0, op0=Alu.max, op1=Alu.min)
                sfc = wt("sfc" + suff, n)
                STT(out=sfc, in0=y0s, scalar=128.0, in1=xbB, op0=Alu.mult, op1=Alu.add)
                nc.vector.tensor_copy(out=sidx_tile[:], in_=sfc)
                return tyc, txc, yf, xf_

            sidxA = wchain.tile([128, KCOLS], i32, tag="sidxA", name="sidxA")
            sidxB = wchain.tile([128, NC_ - KCOLS], i32, tag="sidxB", name="sidxB")

            offA = fast_sb[:, 0:2 * KCOLS].rearrange("p (m c) -> p m c", c=2)
            tyA, txA, y0fA, x0fA = idx_chain(
                KCOLS, offA[:, :, 0], offA[:, :, 1],
                fast_sb[:, 2 * KCOLS:3 * KCOLS], fast_sb[:, 3 * KCOLS:4 * KCOLS],
                fast_sb[:, 4 * KCOLS:4 * KCOLS + 1], sidxA, "A")
            offB = offp_sb.rearrange("p (m c) -> p m c", c=2)
            tyB, txB, y0fB, x0fB = idx_chain(
                NC_ - KCOLS, offB[:, KCOLS:NC_, 0], offB[:, KCOLS:NC_, 1],
                yk_sb[:, KCOLS:NC_], xk_sb[:, KCOLS:NC_], xg_sb[:, 0:1], sidxB, "B")

            # full-width ty/tx/y0f/x0f for the weight chain
            ty = wt("ty"); tx = wt("tx"); y0f = wt("y0f"); x0f = wt("x0f")
            nc.vector.tensor_copy(out=ty[:, 0:KCOLS], in_=tyA)
            nc.vector.tensor_copy(out=ty[:, KCOLS:NC_], in_=tyB)
            nc.vector.tensor_copy(out=tx[:, 0:KCOLS], in_=txA)
            nc.vector.tensor_copy(out=tx[:, KCOLS:NC_], in_=txB)
            nc.vector.tensor_copy(out=y0f[:, 0:KCOLS], in_=y0fA)
            nc.vector.tensor_copy(out=y0f[:, KCOLS:NC_], in_=y0fB)
            nc.vector.tensor_copy(out=x0f[:, 0:KCOLS], in_=x0fA)
            nc.vector.tensor_copy(out=x0f[:, KCOLS:NC_], in_=x0fB)

            fy = wt("fy"); STT(out=fy, in0=ty, scalar=0.5, in1=y0f, op0=Alu.add, op1=Alu.subtract)
            fx = wt("fx"); STT(out=fx, in0=tx, scalar=0.5, in1=x0f, op0=Alu.add, op1=Alu.subtract)

            y0c = wt("y0c"); TS(out=y0c, in0=y0f, scalar1=0.0, scalar2=127.0, op0=Alu.max, op1=Alu.min)
            v0 = wt("v0"); TT(out=v0, in0=y0f, in1=y0c, op=Alu.is_equal)
            y1f = wt("y1f"); TS(out=y1f, in0=y0f, scalar1=1.0, scalar2=None, op0=Alu.add)
            y1c = wt("y1c"); TS(out=y1c, in0=y1f, scalar1=0.0, scalar2=127.0, op0=Alu.max, op1=Alu.min)
            v1 = wt("v1"); TT(out=v1, in0=y1f, in1=y1c, op=Alu.is_equal)

            xc0 = wt("xc0"); TS(out=xc0, in0=x0f, scalar1=0.0, scalar2=127.0, op0=Alu.max, op1=Alu.min)
            vx0 = wt("vx0"); TT(out=vx0, in0=x0f, in1=xc0, op=Alu.is_equal)
            x1f = wt("x1f"); TS(out=x1f, in0=x0f, scalar1=1.0, scalar2=None, op0=Alu.add)
            xc1 = wt("xc1"); TS(out=xc1, in0=x1f, scalar1=0.0, scalar2=127.0, op0=Alu.max, op1=Alu.min)
            vx1 = wt("vx1"); TT(out=vx1, in0=x1f, in1=xc1, op=Alu.is_equal)

            wy0 = wt("wy0"); TS(out=wy0, in0=fy, scalar1=-1.0, scalar2=1.0, op0=Alu.mult, op1=Alu.add)
            wy0v = wt("wy0v"); TT(out=wy0v, in0=wy0, in1=v0, op=Alu.mult)
            wy1v = wt("wy1v"); TT(out=wy1v, in0=fy, in1=v1, op=Alu.mult)
            wx0 = wt("wx0"); TS(out=wx0, in0=fx, scalar1=-1.0, scalar2=1.0, op0=Alu.mult, op1=Alu.add)
            wx0v = wt("wx0v"); TT(out=wx0v, in0=wx0, in1=vx0, op=Alu.mult)
            wx1v = wt("wx1v"); TT(out=wx1v, in0=fx, in1=vx1, op=Alu.mult)

            w00 = wt("w00"); TT(out=w00, in0=wy0v, in1=wx0v, op=Alu.mult)
            w01 = wt("w01"); TT(out=w01, in0=wy0v, in1=wx1v, op=Alu.mult)
            w10 = wt("w10"); TT(out=w10, in0=wy1v, in1=wx0v, op=Alu.mult)
            w11 = wt("w11"); TT(out=w11, in0=wy1v, in1=wx1v, op=Alu.mult)
            wq = (w00, w01, w10, w11)

            # ---- main loop ----
            for blk in range(BLKS):
                m = mpool.tile([128, BLKROWS, C], bf16, tag="m", name="m")
                att = opool.tile([G4, BLKROWS, W], f32, tag="att", name="att")
                for k in range(NK):
                    Gq = gpool.tile([128, BLKROWS, 4 * C], bf16, tag="Gq", name="Gq")
                    for y in range(BLKROWS):
                        col = k * ROWS + blk * BLKROWS + y
                        idx_ap = (sidxA[:, col:col + 1] if k == 0
                                  else sidxB[:, col - KCOLS:col - KCOLS + 1])
                        nc.gpsimd.indirect_dma_start(
                            out=Gq[:, y, :], out_offset=None, in_=xf[:, :],
                            in_offset=bass.IndirectOffsetOnAxis(ap=idx_ap, axis=0))
                    for y in range(BLKROWS):
                        col = k * ROWS + blk * BLKROWS + y
                        dq = dpool.tile([128, 4, 128], bf16, tag=f"dq{y % 4}",
                                        name=f"dq{y % 4}")
                        for j in range(4):
                            TS(out=dq[:, j, :], in0=id_sb[:],
                               scalar1=wq[j][:, col:col + 1], scalar2=None,
                               op0=Alu.mult)
                        ps = psum_pool.tile([128, C], f32, tag=f"ps{y % 4}",
                                            name=f"ps{y % 4}")
                        for j in range(4):
                            nc.tensor.matmul(
                                out=ps[:, :], lhsT=dq[:, j, :],
                                rhs=Gq[:, y, j * C:(j + 1) * C],
                                start=(j == 0), stop=(j == 3))
                        # ACT (otherwise idle) evicts PSUM f32 -> SBUF bf16 so
                        # the DVE max runs in the 2x bf16 mode.
                        if k == 0:
                            nc.scalar.activation(
                                out=m[:, y, :], in_=ps[:, :],
                                func=mybir.ActivationFunctionType.Copy)
                        else:
                            es = spool.tile([128, C], bf16, tag=f"es{y % 4}",
                                            name=f"es{y % 4}")
                            nc.scalar.activation(
                                out=es[:, :], in_=ps[:, :],
                                func=mybir.ActivationFunctionType.Copy)
                            TT(out=m[:, y, :], in0=m[:, y, :], in1=es[:, :],
                               op=Alu.max)
                        # interleave the A-contraction with the last kernel
                        # point so only the final row's tail is exposed:
                        # PE transposes m row -> mT, PE contracts with A^T,
                        # ACT applies sigmoid(logits + c0) straight from PSUM.
                        if k == NK - 1:
                            psT = psum_pool.tile([128, 2, 128], bf16,
                                                 tag=f"psT{y % 2}", name=f"psT{y % 2}")
                            for h in range(2):
                                nc.tensor.transpose(
                                    out=psT[:, h, :],
                                    in_=m[:, y, h * 128:(h + 1) * 128],
                                    identity=id_sb[:])
                            mT = spool.tile([128, 2, 128], bf16, tag=f"mT{y % 2}",
                                            name=f"mT{y % 2}")
                            nc.vector.tensor_copy(out=mT[:], in_=psT[:])
                            psL = psum_pool.tile([G4, 128], f32, tag=f"psL{y % 2}",
                                                 name=f"psL{y % 2}")
                            for h in range(2):
                                nc.tensor.matmul(
                                    out=psL[:, :], lhsT=at_sb[:, h * G4:(h + 1) * G4],
                                    rhs=mT[:, h, :], start=(h == 0), stop=(h == 1))
                            nc.scalar.activation(
                                out=att[:, y, :], in_=psL[:, :],
                                func=mybir.ActivationFunctionType.Sigmoid,
                                bias=c0_sb[:, 0:1])

                # ---- store [4,16,W] block ----
                dst = bass.AP(tensor=out, offset=blk * BLKROWS * W,
                              ap=[[ROWS * W, G4], [W, BLKROWS], [1, W]])
                nc.sync.dma_start(out=dst, in_=att[:, :, :])

    _split_waits(nc)
    return nc


def _marshal(inputs):
    x = np.ascontiguousarray(inputs["x"], dtype=np.float32)
    offset = np.ascontiguousarray(inputs["offset"], dtype=np.float32)
    W0 = np.asarray(inputs["W0"], np.float32); b0 = np.asarray(inputs["b0"], np.float32)
    gamma = np.asarray(inputs["gamma"], np.float32); beta = np.asarray(inputs["beta"], np.float32)
    rm = np.asarray(inputs["run_mean"], np.float32); rv = np.asarray(inputs["run_var"], np.float32)
    W1 = np.asarray(inputs["W1"], np.float32); b1 = np.asarray(inputs["b1"], np.float32)

    inv = gamma / np.sqrt(rv + BN_EPS)
    A = (W1 * inv[None, :]) @ W0              # (4, 256)
    c0 = W1 @ (inv * (b0 - rm) + beta) + b1   # (4,)

    # atp[c', h*4+g] = A[g, h*128+c']  (A^T in two 128-channel chunks)
    atm = A.reshape(G4, 2, 128).transpose(2, 1, 0).reshape(128, 2 * G4)
    atm = np.ascontiguousarray(atm.astype(ml_dtypes.bfloat16))
    c0col = np.ascontiguousarray(c0.reshape(G4, 1).astype(np.float32))
    xgrid = np.arange(128, dtype=np.float32).reshape(128, 1).copy()
    idmat = np.eye(128, dtype=ml_dtypes.bfloat16)

    ky = np.repeat(np.arange(-1, 2), 3).astype(np.float32)   # k//3 - 1
    kx = np.tile(np.arange(-1, 2), 3).astype(np.float32)     # k%3 - 1

    NT = 16788  # table rows: s in [0, (128+2)*128+129 + pad]
    xf_b = []
    for b in range(B):
        # F' = image rows -2..129 zero-padded, +1 position shift (xb1 = x0+1+1)
        Ff = np.zeros((132 * W + 2 + 130, C), ml_dtypes.bfloat16)
        Ff[2 * W + 1:2 * W + 1 + NPOS] = x[b].transpose(1, 2, 0).reshape(NPOS, C).astype(ml_dtypes.bfloat16)
        T = np.concatenate([Ff[0:NT], Ff[1:NT + 1], Ff[W:NT + W], Ff[W + 1:NT + W + 1]], axis=1)
        xf_b.append(np.ascontiguousarray(T))

    in_maps = []
    for core in range(NCORES):
        b = core // 4
        r0 = (core % 4) * ROWS
        # off_px[x, k, y, c] = offset[b, 2k+c, r0+y, x]
        off = offset[b].reshape(NK, 2, H, W)[:, :, r0:r0 + ROWS, :]
        off_px = off.transpose(3, 0, 2, 1).reshape(128, NK * ROWS * 2).copy()
        yv = np.arange(r0, r0 + ROWS, dtype=np.float32)
        ykc = (yv[None, :] + ky[:, None] - 0.5).reshape(1, NK * ROWS)
        ykc = np.broadcast_to(ykc, (128, NK * ROWS)).astype(np.float32).copy()
        xkc = np.broadcast_to((kx[:, None] - 0.5) * np.ones((1, ROWS), np.float32),
                              (NK, ROWS)).reshape(1, NK * ROWS)
        xkc = np.broadcast_to(xkc, (128, NK * ROWS)).astype(np.float32).copy()
        fast = np.concatenate([off_px[:, 0:64], ykc[:, 0:32], xkc[:, 0:32], xgrid],
                              axis=1).astype(np.float32)
        big = np.concatenate([off_px, ykc, xkc, xgrid], axis=1).astype(np.float32)
        in_maps.append(dict(xf=xf_b[b], fastp=np.ascontiguousarray(fast),
                            bigp=np.ascontiguousarray(big),
                            atp=atm, c0t=c0col, idp=idmat))
    return in_maps


def kernel(**inputs):
    if "nc" not in _prog_cache:
        _prog_cache["nc"] = _build_program()
    nc = _prog_cache["nc"]
    in_maps = _marshal(inputs)
    res = run_bass_kernel_spmd(nc, in_maps, list(range(NCORES)))
    out = np.zeros((B, C, H, W), np.float32)
    for core in range(NCORES):
        b = core // 4
        r0 = (core % 4) * ROWS
        att = res.results[core]["out"]                      # (4, 32, 128)
        out[b, :, r0:r0 + ROWS, :] = np.tile(att, (C // G4, 1, 1))
    return out


# revision 35
# speedup vs baseline: 1.5021x; 1.0016x over previous
# Trainium2 Bass kernel for nn_DeformSpaceAttentionv2 (deformable 3x3 max-
# sampling attention). Self-contained: hardcodes all shapes/sharding.
#
# Math: the whole channel pipeline after the deformable-unfold-max collapses
# to logits = A @ feat + c0 with A = W1*diag(gamma/sqrt(var+eps))*W0 (4x256),
# so per pixel we need feat[c] = max_k bilinear_k(x)[c], then a 4-way
# contraction, sigmoid, and channel-tiling (done host-side: pure replication).
#
# Sharding: 8 cores = batch (2) x 32-row bands (4). Per core:
#  - GPSIMD issues 288 one-index-per-partition indirect gathers (9 kernel
#    points x 32 rows) from a precomputed 4-corner neighborhood table in HBM
#    (T[s] = x-channels at positions s, s+1, s+128, s+129 of the zero-padded
#    image; 1024 bf16 elems/row). This is the kernel's pacing floor: SWDGE
#    descriptor-gen costs ~1.04us/gather of Pool engine time, indirect DMA is
#    gpsimd-only, and multi-index / dma_gather batched forms were probed
#    broken on this PJRT path (multi-index: second index ignored, partitions
#    >0 corrupt; dma_gather: needs a Q7 library that can't load here). The
#    gathers run gapless; every other engine hides underneath them.
#  - Vector engine computes bilinear corner weights / validity / gather
#    indices (floor via round-to-nearest cast tricks). The k=0 index columns
#    are computed first from a tiny 'fastp' const (first HWDGE slot) so the
#    gather stream launches ~5us into the kernel.
#  - PE does the bilinear corner MAC: per (k,y) the per-pixel corner weight
#    is placed on the diagonal of a 128x128 stationary matrix (built with a
#    single 4x-mode tensor_scalar vs the identity), and 4 accumulating
#    matmuls (one per corner) compute sample = sum_j diag(w_j) @ G_j into
#    PSUM. This moves the whole multiply-add load off DVE (the baseline
#    bottleneck at 94% busy; TSP runs 4x, but STT/TT adds only 1x/2x).
#  - ACT evicts samples PSUM f32 -> SBUF bf16; DVE runs the 9-way max in the
#    2x bf16 TT mode (last kernel point: straight from PSUM, one hop less).
#  - Tail rides PE too: per row, PE transposes m to channel-partitions, PE
#    contracts with A^T into [4,W] logits, ACT applies sigmoid with c0 as
#    the per-partition activation bias, stores go out per half-block. The
#    64x channel replication happens on host (pure memory duplication).
import numpy as np
import ml_dtypes

import concourse.bass as bass
import concourse.tile as tile
from concourse import mybir
from concourse.bass_utils import run_bass_kernel_spmd

BN_EPS = 1e-5
B, C, H, W = 2, 256, 128, 128
G4 = 4
ROWS = 32            # output rows per core
NCORES = 8
NPOS = H * W         # 16384
NK = 9
BLKS = 2             # 16-row blocks per core
BLKROWS = 16

f32 = mybir.dt.float32
bf16 = mybir.dt.bfloat16
i16 = mybir.dt.int16
i32 = mybir.dt.int32

_prog_cache = {}


def _split_waits(nc, max_waits=1):
    """walrus codegen supports only 1 sem-wait per instruction; split extras
    onto preceding NoOps."""
    for bb in nc.m.functions[0].blocks:
        new_insts = []
        for ins in bb.instructions:
            si = ins.sync_info
            if si is not None and si.on_wait and len(si.on_wait) > max_waits:
                waits = list(si.on_wait)
                extra, keep = waits[:-max_waits], waits[-max_waits:]
                for i in range(0, len(extra), max_waits):
                    chunk = extra[i:i + max_waits]
                    nop = mybir.InstNoOp(name=f"{ins.name}-wsplit-{i}", ins=[], outs=[])
                    nop.engine = ins.engine
                    nop.sync_info = mybir.SyncInfo(on_wait=chunk, on_update=[])
                    new_insts.append(nop)
                si.on_wait = keep
            new_insts.append(ins)
        bb.instructions[:] = new_insts


def _build_program():
    nc = bass.Bass("TRN2", target_bir_lowering=False)

    xf = nc.declare_dram_parameter("xf", [16788, 4 * C], bf16, isOutput=False)
    # fastp = [off k=0 (64) | yk k=0 (32) | xk k=0 (32) | xg (1)] - the minimal
    # inputs for the k=0 index chain, loaded in the first HWDGE slot.
    fastp = nc.declare_dram_parameter("fastp", [128, 129], f32, isOutput=False)
    # bigp = [offp (576) | yk (288) | xk (288)] - everything else, one DMA.
    bigp = nc.declare_dram_parameter("bigp", [128, NK * ROWS * 4 + 1], f32, isOutput=False)
    atp = nc.declare_dram_parameter("atp", [128, 2 * G4], bf16, isOutput=False)
    c0t = nc.declare_dram_parameter("c0t", [G4, 1], f32, isOutput=False)
    idp = nc.declare_dram_parameter("idp", [128, 128], bf16, isOutput=False)
    out = nc.declare_dram_parameter("out", [G4, ROWS, W], f32, isOutput=True)

    NC_ = NK * ROWS          # 288 weight columns
    with tile.TileContext(nc) as tc:
        with (
            tc.tile_pool(name="consts", bufs=1) as consts,
            tc.tile_pool(name="wchain", bufs=1) as wchain,
            tc.tile_pool(name="gpool", bufs=2) as gpool,
            tc.tile_pool(name="dpool", bufs=4) as dpool,
            tc.tile_pool(name="mpool", bufs=2) as mpool,
            tc.tile_pool(name="spool", bufs=3) as spool,
            tc.tile_pool(name="opool", bufs=2) as opool,
            tc.tile_pool(name="psum", bufs=1, space="PSUM") as psum_pool,
        ):
            # ---- const loads: fastp grabs the first HWDGE slot so the k=0
            # index chain (and hence the Pool gather stream) starts ASAP ----
            fast_sb = consts.tile([128, 129], f32)
            nc.sync.dma_start(out=fast_sb, in_=fastp[:, :])
            big_sb = consts.tile([128, NK * ROWS * 4 + 1], f32)
            nc.sync.dma_start(out=big_sb, in_=bigp[:, :])
            at_sb = consts.tile([128, 2 * G4], bf16)
            nc.scalar.dma_start(out=at_sb, in_=atp[:, :])
            c0_sb = consts.tile([G4, 1], f32)
            nc.scalar.dma_start(out=c0_sb, in_=c0t[:, :])
            id_sb = consts.tile([128, 128], bf16)
            nc.scalar.dma_start(out=id_sb, in_=idp[:, :])
            offp_sb = big_sb[:, 0:NC_ * 2]
            yk_sb = big_sb[:, NC_ * 2:NC_ * 3]
            xk_sb = big_sb[:, NC_ * 3:NC_ * 4]
            xg_sb = big_sb[:, NC_ * 4:NC_ * 4 + 1]

            Alu = mybir.AluOpType
            TT = nc.vector.tensor_tensor
            TS = nc.vector.tensor_scalar
            STT = nc.vector.scalar_tensor_tensor

            def wt(name, cols=NC_):
                return wchain.tile([128, cols], f32, tag=name, name=name)

            # ---- weight / index chain ----
            # Index chain in two stages: stage A covers k=0 only, reading the
            # small fastp const (lands ~2us), so gathers start ASAP.
            KCOLS = ROWS  # 32 cols per kernel point

            def idx_chain(n, offy_ap, offx_ap, yk_ap, xk_ap, xg_ap, sidx_tile, suff):
                tyc = wt("tyc" + suff, n)
                TT(out=tyc, in0=offy_ap, in1=yk_ap, op=Alu.add)
                txc0 = wt("txc0" + suff, n)
                TT(out=txc0, in0=offx_ap, in1=xk_ap, op=Alu.add)
                txc = wt("txc" + suff, n)
                TS(out=txc, in0=txc0, scalar1=xg_ap, scalar2=None, op0=Alu.add)
                yi = wchain.tile([128, n], i32, tag="yi" + suff, name="yi" + suff)
                nc.vector.tensor_copy(out=yi, in_=tyc)       # rne(py-0.5)=floor(py)
                yf = wt("yf" + suff, n)
                nc.vector.tensor_copy(out=yf, in_=yi)
                xi = wchain.tile([128, n], i32, tag="xi" + suff, name="xi" + suff)
                nc.vector.tensor_copy(out=xi, in_=txc)
                xf_ = wt("xf" + suff, n)
                nc.vector.tensor_copy(out=xf_, in_=xi)
                # xb2 = clip(x0, -1, 128) + 257 = clip(x0 + 257, 256, 385)
                xbA = wt("xbA" + suff, n)
                TS(out=xbA, in0=xf_, scalar1=257.0, scalar2=256.0, op0=Alu.add, op1=Alu.max)
                xbB = wt("xbB" + suff, n)
                TS(out=xbB, in0=xbA, scalar1=385.0, scalar2=None, op0=Alu.min)
                y0s = wt("y0s" + suff, n)
                TS(out=y0s, in0=yf, scalar1=-2.0, scalar2=128.0, op0=Alu.max, op1=Alu.min)
                sfc = wt("sfc" + suff, n)
                STT(out=sfc, in0=y0s, scalar=128.0, in1=xbB, op0=Alu.mult, op1=Alu.add)
                nc.vector.tensor_copy(out=sidx_tile[:], in_=sfc)
                return tyc, txc, yf, xf_

            sidxA = wchain.tile([128, KCOLS], i32, tag="sidxA", name="sidxA")
            sidxB = wchain.tile([128, NC_ - KCOLS], i32, tag="sidxB", name="sidxB")

            offA = fast_sb[:, 0:2 * KCOLS].rearrange("p (m c) -> p m c", c=2)
            tyA, txA, y0fA, x0fA = idx_chain(
                KCOLS, offA[:, :, 0], offA[:, :, 1],
                fast_sb[:, 2 * KCOLS:3 * KCOLS], fast_sb[:, 3 * KCOLS:4 * KCOLS],
                fast_sb[:, 4 * KCOLS:4 * KCOLS + 1], sidxA, "A")
            offB = offp_sb.rearrange("p (m c) -> p m c", c=2)
            tyB, txB, y0fB, x0fB = idx_chain(
                NC_ - KCOLS, offB[:, KCOLS:NC_, 0], offB[:, KCOLS:NC_, 1],
                yk_sb[:, KCOLS:NC_], xk_sb[:, KCOLS:NC_], xg_sb[:, 0:1], sidxB, "B")

            # full-width ty/tx/y0f/x0f for the weight chain
            ty = wt("ty"); tx = wt("tx"); y0f = wt("y0f"); x0f = wt("x0f")
            nc.vector.tensor_copy(out=ty[:, 0:KCOLS], in_=tyA)
            nc.vector.tensor_copy(out=ty[:, KCOLS:NC_], in_=tyB)
            nc.vector.tensor_copy(out=tx[:, 0:KCOLS], in_=txA)
            nc.vector.tensor_copy(out=tx[:, KCOLS:NC_], in_=txB)
            nc.vector.tensor_copy(out=y0f[:, 0:KCOLS], in_=y0fA)
            nc.vector.tensor_copy(out=y0f[:, KCOLS:NC_], in_=y0fB)
            nc.vector.tensor_copy(out=x0f[:, 0:KCOLS], in_=x0fA)
            nc.vector.tensor_copy(out=x0f[:, KCOLS:NC_], in_=x0fB)

            fy = wt("fy"); STT(out=fy, in0=ty, scalar=0.5, in1=y0f, op0=Alu.add, op1=Alu.subtract)
            fx = wt("fx"); STT(out=fx, in0=tx, scalar=0.5, in1=x0f, op0=Alu.add, op1=Alu.subtract)

            y0c = wt("y0c"); TS(out=y0c, in0=y0f, scalar1=0.0, scalar2=127.0, op0=Alu.max, op1=Alu.min)
            v0 = wt("v0"); TT(out=v0, in0=y0f, in1=y0c, op=Alu.is_equal)
            y1f = wt("y1f"); TS(out=y1f, in0=y0f, scalar1=1.0, scalar2=None, op0=Alu.add)
            y1c = wt("y1c"); TS(out=y1c, in0=y1f, scalar1=0.0, scalar2=127.0, op0=Alu.max, op1=Alu.min)
            v1 = wt("v1"); TT(out=v1, in0=y1f, in1=y1c, op=Alu.is_equal)

            xc0 = wt("xc0"); TS(out=xc0, in0=x0f, scalar1=0.0, scalar2=127.0, op0=Alu.max, op1=Alu.min)
            vx0 = wt("vx0"); TT(out=vx0, in0=x0f, in1=xc0, op=Alu.is_equal)
            x1f = wt("x1f"); TS(out=x1f, in0=x0f, scalar1=1.0, scalar2=None, op0=Alu.add)
            xc1 = wt("xc1"); TS(out=xc1, in0=x1f, scalar1=0.0, scalar2=127.0, op0=Alu.max, op1=Alu.min)
            vx1 = wt("vx1"); TT(out=vx1, in0=x1f, in1=xc1, op=Alu.is_equal)

            wy0 = wt("wy0"); TS(out=wy0, in0=fy, scalar1=-1.0, scalar2=1.0, op0=Alu.mult, op1=Alu.add)
            wy0v = wt("wy0v"); TT(out=wy0v, in0=wy0, in1=v0, op=Alu.mult)
            wy1v = wt("wy1v"); TT(out=wy1v, in0=fy, in1=v1, op=Alu.mult)
            wx0 = wt("wx0"); TS(out=wx0, in0=fx, scalar1=-1.0, scalar2=1.0, op0=Alu.mult, op1=Alu.add)
            wx0v = wt("wx0v"); TT(out=wx0v, in0=wx0, in1=vx0, op=Alu.mult)
            wx1v = wt("wx1v"); TT(out=wx1v, in0=fx, in1=vx1, op=Alu.mult)

            w00 = wt("w00"); TT(out=w00, in0=wy0v, in1=wx0v, op=Alu.mult)
            w01 = wt("w01"); TT(out=w01, in0=wy0v, in1=wx1v, op=Alu.mult)
            w10 = wt("w10"); TT(out=w10, in0=wy1v, in1=wx0v, op=Alu.mult)
            w11 = wt("w11"); TT(out=w11, in0=wy1v, in1=wx1v, op=Alu.mult)
            wq = (w00, w01, w10, w11)

            # ---- main loop ----
            for blk in range(BLKS):
                m = mpool.tile([128, BLKROWS, C], bf16, tag="m", name="m")
                # two half-block att tiles so the first store dispatches while
                # the second half is still finishing
                atts = [opool.tile([G4, BLKROWS // 2, W], f32, tag=f"att{h}",
                                   name=f"att{h}") for h in range(2)]
                for k in range(NK):
                    Gq = gpool.tile([128, BLKROWS, 4 * C], bf16, tag="Gq", name="Gq")
                    for y in range(BLKROWS):
                        col = k * ROWS + blk * BLKROWS + y
                        idx_ap = (sidxA[:, col:col + 1] if k == 0
                                  else sidxB[:, col - KCOLS:col - KCOLS + 1])
                        nc.gpsimd.indirect_dma_start(
                            out=Gq[:, y, :], out_offset=None, in_=xf[:, :],
                            in_offset=bass.IndirectOffsetOnAxis(ap=idx_ap, axis=0))
                    for y in range(BLKROWS):
                        col = k * ROWS + blk * BLKROWS + y
                        dq = dpool.tile([128, 4, 128], bf16, tag=f"dq{y % 4}",
                                        name=f"dq{y % 4}")
                        for j in range(4):
                            TS(out=dq[:, j, :], in0=id_sb[:],
                               scalar1=wq[j][:, col:col + 1], scalar2=None,
                               op0=Alu.mult)
                        ps = psum_pool.tile([128, C], f32, tag=f"ps{y % 4}",
                                            name=f"ps{y % 4}")
                        for j in range(4):
                            nc.tensor.matmul(
                                out=ps[:, :], lhsT=dq[:, j, :],
                                rhs=Gq[:, y, j * C:(j + 1) * C],
                                start=(j == 0), stop=(j == 3))
                        # ACT (otherwise idle) evicts PSUM f32 -> SBUF bf16 so
                        # the DVE max runs in the 2x bf16 mode. On the last
                        # kernel point DVE maxes straight from PSUM instead -
                        # one hop less on the closing critical path.
                        if k == 0:
                            nc.scalar.activation(
                                out=m[:, y, :], in_=ps[:, :],
                                func=mybir.ActivationFunctionType.Copy)
                        elif k == NK - 1:
                            TT(out=m[:, y, :], in0=m[:, y, :], in1=ps[:, :],
                               op=Alu.max)
                        else:
                            es = spool.tile([128, C], bf16, tag=f"es{y % 4}",
                                            name=f"es{y % 4}")
                            nc.scalar.activation(
                                out=es[:, :], in_=ps[:, :],
                                func=mybir.ActivationFunctionType.Copy)
                            TT(out=m[:, y, :], in0=m[:, y, :], in1=es[:, :],
                               op=Alu.max)
                        # interleave the A-contraction with the last kernel
                        # point so only the final row's tail is exposed:
                        # PE transposes m row -> mT, PE contracts with A^T,
                        # ACT applies sigmoid(logits + c0) straight from PSUM.
                        if k == NK - 1:
                            psT = psum_pool.tile([128, 2, 128], bf16,
                                                 tag=f"psT{y % 2}", name=f"psT{y % 2}")
                            for h in range(2):
                                nc.tensor.transpose(
                                    out=psT[:, h, :],
                                    in_=m[:, y, h * 128:(h + 1) * 128],
                                    identity=id_sb[:])
                            mT = spool.tile([128, 2, 128], bf16, tag=f"mT{y % 2}",
                                            name=f"mT{y % 2}")
                            nc.vector.tensor_copy(out=mT[:], in_=psT[:])
                            psL = psum_pool.tile([G4, 128], f32, tag=f"psL{y % 2}",
                                                 name=f"psL{y % 2}")
                            for h in range(2):
                                nc.tensor.matmul(
                                    out=psL[:, :], lhsT=at_sb[:, h * G4:(h + 1) * G4],
                                    rhs=mT[:, h, :], start=(h == 0), stop=(h == 1))
                            nc.scalar.activation(
                                out=atts[y // 8][:, y % 8, :], in_=psL[:, :],
                                func=mybir.ActivationFunctionType.Sigmoid,
                                bias=c0_sb[:, 0:1])
                            if y % 8 == 7:
                                h = y // 8
                                dst = bass.AP(
                                    tensor=out,
                                    offset=(blk * BLKROWS + h * 8) * W,
                                    ap=[[ROWS * W, G4], [W, BLKROWS // 2], [1, W]])
                                nc.sync.dma_start(out=dst, in_=atts[h][:, :, :])

    _split_waits(nc)
    return nc


def _marshal(inputs):
    x = np.ascontiguousarray(inputs["x"], dtype=np.float32)
    offset = np.ascontiguousarray(inputs["offset"], dtype=np.float32)
    W0 = np.asarray(inputs["W0"], np.float32); b0 = np.asarray(inputs["b0"], np.float32)
    gamma = np.asarray(inputs["gamma"], np.float32); beta = np.asarray(inputs["beta"], np.float32)
    rm = np.asarray(inputs["run_mean"], np.float32); rv = np.asarray(inputs["run_var"], np.float32)
    W1 = np.asarray(inputs["W1"], np.float32); b1 = np.asarray(inputs["b1"], np.float32)

    inv = gamma / np.sqrt(rv + BN_EPS)
    A = (W1 * inv[None, :]) @ W0              # (4, 256)
    c0 = W1 @ (inv * (b0 - rm) + beta) + b1   # (4,)

    # atp[c', h*4+g] = A[g, h*128+c']  (A^T in two 128-channel chunks)
    atm = A.reshape(G4, 2, 128).transpose(2, 1, 0).reshape(128, 2 * G4)
    atm = np.ascontiguousarray(atm.astype(ml_dtypes.bfloat16))
    c0col = np.ascontiguousarray(c0.reshape(G4, 1).astype(np.float32))
    xgrid = np.arange(128, dtype=np.float32).reshape(128, 1).copy()
    idmat = np.eye(128, dtype=ml_dtypes.bfloat16)

    ky = np.repeat(np.arange(-1, 2), 3).astype(np.float32)   # k//3 - 1
    kx = np.tile(np.arange(-1, 2), 3).astype(np.float32)     # k%3 - 1

    NT = 16788  # table rows: s in [0, (128+2)*128+129 + pad]
    xf_b = []
    for b in range(B):
        # F' = image rows -2..129 zero-padded, +1 position shift (xb1 = x0+1+1)
        Ff = np.zeros((132 * W + 2 + 130, C), ml_dtypes.bfloat16)
        Ff[2 * W + 1:2 * W + 1 + NPOS] = x[b].transpose(1, 2, 0).reshape(NPOS, C).astype(ml_dtypes.bfloat16)
        T = np.concatenate([Ff[0:NT], Ff[1:NT + 1], Ff[W:NT + W], Ff[W + 1:NT + W + 1]], axis=1)
        xf_b.append(np.ascontiguousarray(T))

    in_maps = []
    for core in range(NCORES):
        b = core // 4
        r0 = (core % 4) * ROWS
        # off_px[x, k, y, c] = offset[b, 2k+c, r0+y, x]
        off = offset[b].reshape(NK, 2, H, W)[:, :, r0:r0 + ROWS, :]
        off_px = off.transpose(3, 0, 2, 1).reshape(128, NK * ROWS * 2).copy()
        yv = np.arange(r0, r0 + ROWS, dtype=np.float32)
        ykc = (yv[None, :] + ky[:, None] - 0.5).reshape(1, NK * ROWS)
        ykc = np.broadcast_to(ykc, (128, NK * ROWS)).astype(np.float32).copy()
        xkc = np.broadcast_to((kx[:, None] - 0.5) * np.ones((1, ROWS), np.float32),
                              (NK, ROWS)).reshape(1, NK * ROWS)
        xkc = np.broadcast_to(xkc, (128, NK * ROWS)).astype(np.float32).copy()
        fast = np.concatenate([off_px[:, 0:64], ykc[:, 0:32], xkc[:, 0:32], xgrid],
                              axis=1).astype(np.float32)
        big = np.concatenate([off_px, ykc, xkc, xgrid], axis=1).astype(np.float32)
        in_maps.append(dict(xf=xf_b[b], fastp=np.ascontiguousarray(fast),
                            bigp=np.ascontiguousarray(big),
                            atp=atm, c0t=c0col, idp=idmat))
    return in_maps


def kernel(**inputs):
    if "nc" not in _prog_cache:
        _prog_cache["nc"] = _build_program()
    nc = _prog_cache["nc"]
    in_maps = _marshal(inputs)
    res = run_bass_kernel_spmd(nc, in_maps, list(range(NCORES)))
    out = np.zeros((B, C, H, W), np.float32)
    for core in range(NCORES):
        b = core // 4
        r0 = (core % 4) * ROWS
        att = res.results[core]["out"]                      # (4, 32, 128)
        out[b, :, r0:r0 + ROWS, :] = np.tile(att, (C // G4, 1, 1))
    return out
